# revision 11
# baseline (speedup 1.0000x reference)
"""Trainium2 Bass kernel for nn_BertMoELayer (B=2,S=2048,D=768,F=3072,E=8,top-2).

Strategy: expert-parallel across 8 NeuronCores (1 expert per core).
Each core receives the full token set, computes the router (fp32), selects
the tokens routed to its expert (top-2 membership), compacts their indices
on-device (sparse_gather), gathers the token rows (dma_gather), runs the
expert FFN in fp32r (PE fast mode), scales by the combine weight, and
scatter-adds the rows into a zeroed output buffer (dma_scatter_add).
Host sums the 8 partial outputs.

Self-contained: hardcodes all shapes; only imports the installed concourse
stack from /opt/trn_rl_repo.
"""
import sys

sys.path.insert(0, "/opt/trn_rl_repo")

import numpy as np

import concourse.bass as bass
import concourse.tile as tile
from concourse import bacc, mybir
from concourse.bass import ds, ts
from concourse.bass_utils import run_bass_kernel_spmd

# Problem shapes
B, S, D, F, E = 2, 2048, 768, 3072, 8
T = B * S                 # 4096 tokens
CAP = 1536                # per-expert slot capacity (expected load 1024)
TPAD = T + CAP            # token rows incl. junk region for sentinel slots
DC = D // 128             # 6 contraction chunks for up-proj
FC = F // 128             # 24 contraction chunks for down-proj
NT = T // 128             # 32 token tiles
NCH = 3                   # FFN slot chunks
CHS = CAP // NCH          # 512 slots per chunk
NG = 8                    # gate groups of 512 tokens
CAND_F = (T + CAP) // 16  # 352 candidate free-dim
SENT_F = T // 16          # 256: sentinel region starts here
CAPF = CAP // 16          # 96
HW_ = D // 2              # 384: down-proj half width

F32 = mybir.dt.float32
F32R = mybir.dt.float32r
I16 = mybir.dt.int16
U32 = mybir.dt.uint32
ALU = mybir.AluOpType
AXX = mybir.AxisListType
ACT = mybir.ActivationFunctionType


def build_program():
    nc = bacc.Bacc("TRN2", target_bir_lowering=False, debug=False)

    x_pad = nc.dram_tensor("x_pad", (TPAD, D), F32, kind="ExternalInput")
    gwc = nc.dram_tensor("gwc", (D, E), F32, kind="ExternalInput")
    wup = nc.dram_tensor("wup", (D, F), F32R, kind="ExternalInput")
    bup = nc.dram_tensor("bup", (F,), F32, kind="ExternalInput")
    # wdn_r[half*FC + m] = w_down[m*128:(m+1)*128, half*384:(half+1)*384]
    wdn_r = nc.dram_tensor("wdn_r", (2 * FC, 128, HW_), F32R,
                           kind="ExternalInput")
    bdn = nc.dram_tensor("bdn", (D,), F32R, kind="ExternalInput")
    ids = nc.dram_tensor("ids", (128, NT), F32, kind="ExternalInput")
    ident = nc.dram_tensor("ident", (128, 128), F32, kind="ExternalInput")
    ones = nc.dram_tensor("ones", (1, 128), F32R, kind="ExternalInput")
    out_pad = nc.dram_tensor("out_pad", (TPAD, D), F32, kind="ExternalOutput")

    with tile.TileContext(nc) as tc:
        with (
            tc.tile_pool(name="const", bufs=1) as const_pool,
            tc.tile_pool(name="dram", bufs=1, space="DRAM") as dram_pool,
            tc.tile_pool(name="route", bufs=1) as route_pool,
        ):
            # ---- constants / small inputs ----
            ident_sb = const_pool.tile([128, 128], F32)
            nc.sync.dma_start(ident_sb[:], ident[:])
            gwc_sb = const_pool.tile([128, DC, E], F32)
            nc.sync.dma_start(gwc_sb[:], gwc.rearrange("(kc p) e -> p kc e", p=128))
            ids_sb = const_pool.tile([128, NT], F32)
            nc.sync.dma_start(ids_sb[:], ids[:])
            bup_sb = const_pool.tile([128, FC], F32)
            nc.sync.dma_start(bup_sb[:], bup.rearrange("(m p) -> p m", p=128))
            bdn_sb = const_pool.tile([1, D], F32R)
            nc.sync.dma_start(bdn_sb[:], bdn[None, :])
            ones_sb = const_pool.tile([1, 128], F32R)
            nc.sync.dma_start(ones_sb[:], ones[:])

            # ---- resident up-proj weights ----
            wup_sb = const_pool.tile([128, DC, F], F32R)
            nc.sync.dma_start(wup_sb[:], wup.rearrange("(kc p) f -> p kc f", p=128))

            # ---- routing products (survive into the FFN phase) ----
            idx_rep = route_pool.tile([128, CAPF], I16)
            cw_sl = route_pool.tile([128, CAP // 128], F32)

            # =========== GATE PHASE ===========
            with (
                tc.tile_pool(name="gzero", bufs=1) as gz_pool,
                tc.tile_pool(name="gin", bufs=2) as gin_pool,
                tc.tile_pool(name="gxt", bufs=2) as gxt_pool,
                tc.tile_pool(name="glt", bufs=2) as glt_pool,
                tc.tile_pool(name="gsoft", bufs=1) as gsoft_pool,
                tc.tile_pool(name="gps_tr", bufs=2, space="PSUM") as gps_tr,
                tc.tile_pool(name="gps_lt", bufs=2, space="PSUM") as gps_lt,
                tc.tile_pool(name="gps_ln", bufs=2, space="PSUM") as gps_ln,
            ):
                # zero the real-token region of out_pad
                zt = gz_pool.tile([128, 4, D], F32)
                nc.any.memset(zt[:], 0.0)
                out_zv = out_pad[0:T, :].rearrange("(n p) d -> p n d", p=128)
                for z in range(NT // 4):
                    nc.sync.dma_start(out_zv[:, ts(z, 4), :], zt[:])

                logits_sb = gsoft_pool.tile([128, NT, E], F32)
                for g in range(NG):
                    xg_nat = gin_pool.tile([128, 4, D], F32, tag="xg")
                    nc.sync.dma_start(
                        xg_nat[:],
                        x_pad[g * 512:(g + 1) * 512, :].rearrange(
                            "(j p) d -> p j d", p=128
                        ),
                    )
                    xT_g = gxt_pool.tile([128, DC, 512], F32, tag="xT")
                    for j in range(4):
                        for kc in range(DC):
                            pt = gps_tr.tile([128, 128], F32, tag="tr")
                            nc.tensor.transpose(
                                pt[:], xg_nat[:, j, ts(kc, 128)], ident_sb[:]
                            )
                            nc.any.tensor_copy(
                                xT_g[:, kc, ds(j * 128, 128)], pt[:]
                            )
                    lps = gps_lt.tile([8, 512], F32, tag="lt")
                    for kc in range(DC):
                        nc.tensor.matmul(
                            lps[:], gwc_sb[:, kc, :], xT_g[:, kc, :],
                            start=(kc == 0), stop=(kc == DC - 1),
                        )
                    lT_sb = glt_pool.tile([8, 512], F32, tag="lT")
                    nc.any.tensor_copy(lT_sb[:], lps[:])
                    for j in range(4):
                        t = g * 4 + j
                        pn = gps_ln.tile([128, 8], F32, tag="ln")
                        nc.tensor.transpose(
                            pn[:], lT_sb[:, ts(j, 128)], ident_sb[0:8, 0:8]
                        )
                        nc.any.tensor_copy(logits_sb[:, t, :], pn[:])

                # ---- batched softmax + top-2 over all 32 tiles ----
                m1 = gsoft_pool.tile([128, NT], F32)
                nc.vector.tensor_reduce(m1[:], logits_sb[:], AXX.X, ALU.max)
                smx = gsoft_pool.tile([128, NT, E], F32)
                for e in range(E):
                    nc.vector.tensor_sub(
                        smx[:, :, e], logits_sb[:, :, e], m1[:]
                    )
                nc.scalar.activation(
                    smx[:].rearrange("p a b -> p (a b)"),
                    smx[:].rearrange("p a b -> p (a b)"), ACT.Exp,
                )
                zsum = gsoft_pool.tile([128, NT], F32)
                nc.vector.tensor_reduce(zsum[:], smx[:], AXX.X, ALU.add)
                rz = gsoft_pool.tile([128, NT], F32)
                nc.vector.reciprocal(rz[:], zsum[:])
                gt8 = gsoft_pool.tile([128, NT, E], F32)
                for e in range(E):
                    nc.vector.tensor_tensor(
                        gt8[:, :, e], logits_sb[:, :, e], logits_sb[:, :, 0],
                        op=ALU.is_gt,
                    )
                cnt = gsoft_pool.tile([128, NT], F32)
                nc.vector.tensor_reduce(cnt[:], gt8[:], AXX.X, ALU.add)
                mask = gsoft_pool.tile([128, NT], F32)
                nc.vector.tensor_scalar(mask[:], cnt[:], 1.5, None, op0=ALU.is_lt)
                mm1 = gsoft_pool.tile([128, NT], F32)
                nc.vector.tensor_scalar_add(mm1[:], mask[:], -1.0)
                cw0 = gsoft_pool.tile([128, NT], F32)
                nc.vector.tensor_tensor(cw0[:], smx[:, :, 0], rz[:], op=ALU.mult)
                cand_id = gsoft_pool.tile([128, NT], F32)
                cand_cw = gsoft_pool.tile([128, NT], F32)
                nc.vector.tensor_tensor(cand_cw[:], cw0[:], mask[:], op=ALU.mult)
                nc.vector.tensor_add(cand_cw[:], cand_cw[:], mm1[:])
                nc.vector.tensor_tensor(cand_id[:], ids_sb[:], mask[:], op=ALU.mult)
                nc.vector.tensor_add(cand_id[:], cand_id[:], mm1[:])

                # ---- compaction ----
                # regroup [128,32] (p=t%128, i=t//128) -> [16,256] (t%16,t//16)
                scr_id = dram_pool.tile([T], F32, tag="scr_id")
                scr_cw = dram_pool.tile([T], F32, tag="scr_cw")
                nc.gpsimd.dma_start(
                    scr_id[:].rearrange("(i p) -> p i", p=128), cand_id[:]
                )
                nc.gpsimd.dma_start(
                    scr_cw[:].rearrange("(i p) -> p i", p=128), cand_cw[:]
                )
                cand16_id = gsoft_pool.tile([16, CAND_F], F32)
                cand16_cw = gsoft_pool.tile([16, CAND_F], F32)
                nc.gpsimd.dma_start(
                    cand16_id[:, 0:SENT_F],
                    scr_id[:].rearrange("(f b) -> b f", b=16),
                )
                nc.gpsimd.dma_start(
                    cand16_cw[:, 0:SENT_F],
                    scr_cw[:].rearrange("(f b) -> b f", b=16),
                )
                # sentinel candidates: token T (junk row), weight 0
                nc.any.memset(cand16_id[:, SENT_F:CAND_F], float(T))
                nc.any.memset(cand16_cw[:, SENT_F:CAND_F], 0.0)

                # output sized = input so compaction can never overflow; only
                # the first CAPF free-columns (1536 slots) are used downstream.
                sg_id = gsoft_pool.tile([16, CAND_F], F32)
                sg_cw = gsoft_pool.tile([16, CAND_F], F32)
                nf1 = gsoft_pool.tile([1, 1], U32)
                nf2 = gsoft_pool.tile([1, 1], U32)
                nc.gpsimd.sparse_gather(sg_id[:], cand16_id[:], num_found=nf1[:])
                nc.gpsimd.sparse_gather(sg_cw[:], cand16_cw[:], num_found=nf2[:])

                # int16 + replicate to all 8 16-partition groups
                nc.vector.tensor_copy(idx_rep[0:16, :], sg_id[:, 0:CAPF])
                nc.gpsimd.dma_start(idx_rep[16:32, :], idx_rep[0:16, :])
                nc.gpsimd.dma_start(idx_rep[32:64, :], idx_rep[0:32, :])
                nc.gpsimd.dma_start(idx_rep[64:128, :], idx_rep[0:64, :])

                # combine weights (s%16, s//16) -> slot-major [128, 12]
                scr_cw2 = dram_pool.tile([CAP], F32, tag="scr_cw2")
                nc.gpsimd.dma_start(
                    scr_cw2[:].rearrange("(f b) -> b f", b=16), sg_cw[:, 0:CAPF]
                )
                nc.gpsimd.dma_start(
                    cw_sl[:], scr_cw2[:].rearrange("(j p) -> p j", p=128)
                )

            # =========== FFN PHASE ===========
            with (
                tc.tile_pool(name="fxg", bufs=2) as fxg_pool,
                tc.tile_pool(name="fxt", bufs=2) as fxt_pool,
                tc.tile_pool(name="fh", bufs=1) as fh_pool,
                tc.tile_pool(name="fwd", bufs=6) as fwd_pool,
                tc.tile_pool(name="fy", bufs=1) as fy_pool,
                tc.tile_pool(name="fps_tr", bufs=2, space="PSUM") as fps_tr,
                tc.tile_pool(name="fps_up", bufs=2, space="PSUM") as fps_up,
                tc.tile_pool(name="fps_dn", bufs=4, space="PSUM") as fps_dn,
            ):
                for c in range(NCH):
                    idx_c = idx_rep[:, c * (CHS // 16):(c + 1) * (CHS // 16)]
                    xg = fxg_pool.tile([128, CHS // 128, D], F32, tag="xg")
                    nc.gpsimd.dma_gather(
                        xg[:], x_pad[:], idx_c, num_idxs=CHS,
                        num_idxs_reg=CHS, elem_size=D,
                    )
                    xcT = fxt_pool.tile([128, DC, CHS], F32R, tag="xcT")
                    for j in range(CHS // 128):
                        for kc in range(DC):
                            pt = fps_tr.tile([128, 128], F32, tag="tr")
                            nc.tensor.transpose(
                                pt[:], xg[:, j, ts(kc, 128)], ident_sb[:]
                            )
                            nc.any.tensor_copy(
                                xcT[:, kc, ds(j * 128, 128)], pt[:]
                            )
                    # up-projection + gelu -> h^T [128, FC, CHS]
                    h_sb = fh_pool.tile([128, FC, CHS], F32R, tag="h")
                    for m in range(FC):
                        psu = fps_up.tile([128, CHS], F32, tag="up")
                        for kc in range(DC):
                            nc.tensor.matmul(
                                psu[:],
                                wup_sb[:, kc, ts(m, 128)],
                                xcT[:, kc, :],
                                start=(kc == 0), stop=(kc == DC - 1),
                            )
                        nc.scalar.activation(
                            h_sb[:, m, :], psu[:], ACT.Gelu,
                            bias=bup_sb[:, m:m + 1],
                        )
                    # down-projection, natural orientation, + bias + scale
                    y_lo = fy_pool.tile([128, 2, D], F32, tag="ylo")
                    y_hi = fy_pool.tile([128, 2, D], F32, tag="yhi")
                    y_parts = [y_lo, y_hi]
                    for half in range(2):
                        psd = []
                        for _pi in range(CHS // 128):
                            psd_t = fps_dn.tile([128, HW_], F32, tag="dn")
                            psd.append(psd_t)
                        for m in range(FC):
                            wdn_mh = fwd_pool.tile([128, HW_], F32R, tag="wdn")
                            nc.scalar.dma_start(
                                wdn_mh[:], wdn_r[half * FC + m, :, :]
                            )
                            for blk in range(CHS // 128):
                                nc.tensor.matmul(
                                    psd[blk][:],
                                    h_sb[:, m, ts(blk, 128)],
                                    wdn_mh[:],
                                    start=(m == 0), stop=False,
                                )
                        for blk in range(CHS // 128):
                            nc.tensor.matmul(
                                psd[blk][:],
                                ones_sb[0:1, 0:128],
                                bdn_sb[0:1, ds(half * HW_, HW_)],
                                start=False, stop=True,
                            )
                            nc.vector.tensor_scalar(
                                y_parts[blk // 2][:, blk % 2, ds(half * HW_, HW_)],
                                psd[blk][:],
                                cw_sl[:, c * (CHS // 128) + blk:
                                      c * (CHS // 128) + blk + 1],
                                None,
                                op0=ALU.mult,
                            )
                    for yp in range(2):
                        nc.gpsimd.dma_scatter_add(
                            out_pad[:], y_parts[yp][:],
                            idx_rep[:, (c * 32 + yp * 16):(c * 32 + yp * 16 + 16)],
                            num_idxs=CHS // 2,
                            num_idxs_reg=CHS // 2, elem_size=D,
                        )

    nc.finalize()
    return nc


_NC_CACHE = None


def _get_program():
    global _NC_CACHE
    if _NC_CACHE is None:
        _NC_CACHE = build_program()
    return _NC_CACHE


def make_in_maps(hidden_states, gate_w, w_up, b_up, w_down, b_down):
    hidden_states = np.asarray(hidden_states, dtype=np.float32)
    gate_w = np.asarray(gate_w, dtype=np.float32)
    w_up = np.asarray(w_up, dtype=np.float32)
    b_up = np.asarray(b_up, dtype=np.float32)
    w_down = np.asarray(w_down, dtype=np.float32)
    b_down = np.asarray(b_down, dtype=np.float32)

    x = hidden_states.reshape(T, D)
    x_pad = np.zeros((TPAD, D), dtype=np.float32)
    x_pad[:T] = x
    ids = np.arange(T, dtype=np.float32).reshape(NT, 128).T.copy()  # [128, NT]
    ident = np.eye(128, dtype=np.float32)

    in_maps = []
    for c in range(E):
        gwc = np.concatenate([gate_w[:, c:], gate_w[:, :c]], axis=1).copy()
        wdn = w_down[c]  # [F, D]
        wdn_r = np.ascontiguousarray(
            wdn.reshape(FC, 128, 2, HW_).transpose(2, 0, 1, 3)
        ).reshape(2 * FC, 128, HW_)
        in_maps.append({
            "x_pad": x_pad,
            "gwc": gwc,
            "wup": np.ascontiguousarray(w_up[c]),
            "bup": np.ascontiguousarray(b_up[c]),
            "wdn_r": wdn_r,
            "bdn": np.ascontiguousarray(b_down[c]),
            "ids": ids,
            "ident": ident,
            "ones": np.ones((1, 128), dtype=np.float32),
        })
    return in_maps


def combine_results(results):
    out = np.zeros((T, D), dtype=np.float32)
    for c in range(E):
        out += results[c]["out_pad"][:T]
    return out.reshape(B, S, D)


def kernel(hidden_states, gate_w, w_up, b_up, w_down, b_down):
    in_maps = make_in_maps(hidden_states, gate_w, w_up, b_up, w_down, b_down)
    nc = _get_program()
    res = run_bass_kernel_spmd(nc, in_maps, core_ids=list(range(E)))
    return combine_results(res.results)


if __name__ == "__main__":
    rng = np.random.default_rng(0)
    hs = rng.standard_normal((B, S, D)).astype(np.float32)
    gw = rng.standard_normal((D, E)).astype(np.float32) / np.sqrt(D)
    wu = (rng.standard_normal((E, D, F)) * 0.02).astype(np.float32)
    bu = np.zeros((E, F), dtype=np.float32)
    wd = (rng.standard_normal((E, F, D)) * 0.02).astype(np.float32)
    bd = np.zeros((E, D), dtype=np.float32)
    out = kernel(hs, gw, wu, bu, wd, bd)
    print("out", out.shape, out.dtype, np.abs(out).max())


# revision 12
# speedup vs baseline: 1.0050x; 1.0050x over previous
"""Trainium2 Bass kernel for nn_BertMoELayer (B=2,S=2048,D=768,F=3072,E=8,top-2).

Strategy: expert-parallel across 8 NeuronCores (1 expert per core).
Each core receives the full token set, computes the router (fp32), selects
the tokens routed to its expert (top-2 membership), compacts their indices
on-device (sparse_gather), gathers the token rows (dma_gather), runs the
expert FFN in fp32r (PE fast mode), scales by the combine weight, and
scatter-adds the rows into a zeroed output buffer (dma_scatter_add).
Host sums the 8 partial outputs.

Self-contained: hardcodes all shapes; only imports the installed concourse
stack from /opt/trn_rl_repo.
"""
import sys

sys.path.insert(0, "/opt/trn_rl_repo")

import numpy as np

import concourse.bass as bass
import concourse.tile as tile
from concourse import bacc, mybir
from concourse.bass import ds, ts
from concourse.bass_utils import run_bass_kernel_spmd

# Problem shapes
B, S, D, F, E = 2, 2048, 768, 3072, 8
T = B * S                 # 4096 tokens
CAP = 1536                # per-expert slot capacity (expected load 1024)
TPAD = T + CAP            # token rows incl. junk region for sentinel slots
DC = D // 128             # 6 contraction chunks for up-proj
FC = F // 128             # 24 contraction chunks for down-proj
NT = T // 128             # 32 token tiles
NCH = 3                   # FFN slot chunks
CHS = CAP // NCH          # 512 slots per chunk
NG = 8                    # gate groups of 512 tokens
CAND_F = (T + CAP) // 16  # 352 candidate free-dim
SENT_F = T // 16          # 256: sentinel region starts here
CAPF = CAP // 16          # 96
HW_ = D // 2              # 384: down-proj half width

F32 = mybir.dt.float32
F32R = mybir.dt.float32r
I16 = mybir.dt.int16
U32 = mybir.dt.uint32
ALU = mybir.AluOpType
AXX = mybir.AxisListType
ACT = mybir.ActivationFunctionType


def build_program():
    nc = bacc.Bacc("TRN2", target_bir_lowering=False, debug=False)

    x_pad = nc.dram_tensor("x_pad", (TPAD, D), F32, kind="ExternalInput")
    gwc = nc.dram_tensor("gwc", (D, E), F32, kind="ExternalInput")
    wup = nc.dram_tensor("wup", (D, F), F32R, kind="ExternalInput")
    bup = nc.dram_tensor("bup", (F,), F32, kind="ExternalInput")
    # wdn_r[half*FC + m] = w_down[m*128:(m+1)*128, half*384:(half+1)*384]
    wdn_r = nc.dram_tensor("wdn_r", (2 * FC, 128, HW_), F32R,
                           kind="ExternalInput")
    bdn = nc.dram_tensor("bdn", (D,), F32R, kind="ExternalInput")
    ids = nc.dram_tensor("ids", (128, NT), F32, kind="ExternalInput")
    ident = nc.dram_tensor("ident", (128, 128), F32, kind="ExternalInput")
    ones = nc.dram_tensor("ones", (1, 128), F32R, kind="ExternalInput")
    out_pad = nc.dram_tensor("out_pad", (TPAD, D), F32, kind="ExternalOutput")

    with tile.TileContext(nc) as tc:
        with (
            tc.tile_pool(name="const", bufs=1) as const_pool,
            tc.tile_pool(name="dram", bufs=1, space="DRAM") as dram_pool,
            tc.tile_pool(name="route", bufs=1) as route_pool,
        ):
            # ---- constants / small inputs ----
            ident_sb = const_pool.tile([128, 128], F32)
            nc.sync.dma_start(ident_sb[:], ident[:])
            gwc_sb = const_pool.tile([128, DC, E], F32)
            nc.sync.dma_start(gwc_sb[:], gwc.rearrange("(kc p) e -> p kc e", p=128))
            ids_sb = const_pool.tile([128, NT], F32)
            nc.sync.dma_start(ids_sb[:], ids[:])
            bup_sb = const_pool.tile([128, FC], F32)
            nc.sync.dma_start(bup_sb[:], bup.rearrange("(m p) -> p m", p=128))
            bdn_sb = const_pool.tile([1, D], F32R)
            nc.sync.dma_start(bdn_sb[:], bdn[None, :])
            ones_sb = const_pool.tile([1, 128], F32R)
            nc.sync.dma_start(ones_sb[:], ones[:])

            # ---- resident up-proj weights ----
            wup_sb = const_pool.tile([128, DC, F], F32R)
            nc.sync.dma_start(wup_sb[:], wup.rearrange("(kc p) f -> p kc f", p=128))

            # ---- routing products (survive into the FFN phase) ----
            idx_rep = route_pool.tile([128, CAPF], I16)
            cw_sl = route_pool.tile([128, CAP // 128], F32)

            # =========== GATE PHASE ===========
            with (
                tc.tile_pool(name="gzero", bufs=1) as gz_pool,
                tc.tile_pool(name="gin", bufs=2) as gin_pool,
                tc.tile_pool(name="gxt", bufs=2) as gxt_pool,
                tc.tile_pool(name="glt", bufs=2) as glt_pool,
                tc.tile_pool(name="gsoft", bufs=1) as gsoft_pool,
                tc.tile_pool(name="gps_tr", bufs=2, space="PSUM") as gps_tr,
                tc.tile_pool(name="gps_lt", bufs=2, space="PSUM") as gps_lt,
                tc.tile_pool(name="gps_ln", bufs=2, space="PSUM") as gps_ln,
            ):
                # zero the real-token region of out_pad
                zt = gz_pool.tile([128, 4, D], F32)
                nc.any.memset(zt[:], 0.0)
                out_zv = out_pad[0:T, :].rearrange("(n p) d -> p n d", p=128)
                for z in range(NT // 4):
                    nc.sync.dma_start(out_zv[:, ts(z, 4), :], zt[:])

                logits_sb = gsoft_pool.tile([128, NT, E], F32)
                for g in range(NG):
                    xg_nat = gin_pool.tile([128, 4, D], F32, tag="xg")
                    nc.sync.dma_start(
                        xg_nat[:],
                        x_pad[g * 512:(g + 1) * 512, :].rearrange(
                            "(j p) d -> p j d", p=128
                        ),
                    )
                    xT_g = gxt_pool.tile([128, DC, 512], F32, tag="xT")
                    for j in range(4):
                        for kc in range(DC):
                            pt = gps_tr.tile([128, 128], F32, tag="tr")
                            nc.tensor.matmul(
                                pt[:], xg_nat[:, j, ts(kc, 128)], ident_sb[:]
                            )
                            nc.any.tensor_copy(
                                xT_g[:, kc, ds(j * 128, 128)], pt[:]
                            )
                    lps = gps_lt.tile([8, 512], F32, tag="lt")
                    for kc in range(DC):
                        nc.tensor.matmul(
                            lps[:], gwc_sb[:, kc, :], xT_g[:, kc, :],
                            start=(kc == 0), stop=(kc == DC - 1),
                        )
                    lT_sb = glt_pool.tile([8, 512], F32, tag="lT")
                    nc.any.tensor_copy(lT_sb[:], lps[:])
                    for j in range(4):
                        t = g * 4 + j
                        pn = gps_ln.tile([128, 8], F32, tag="ln")
                        nc.tensor.matmul(
                            pn[:], lT_sb[:, ts(j, 128)], ident_sb[0:8, 0:8]
                        )
                        nc.any.tensor_copy(logits_sb[:, t, :], pn[:])

                # ---- batched softmax + top-2 over all 32 tiles ----
                m1 = gsoft_pool.tile([128, NT], F32)
                nc.vector.tensor_reduce(m1[:], logits_sb[:], AXX.X, ALU.max)
                smx = gsoft_pool.tile([128, NT, E], F32)
                for e in range(E):
                    nc.vector.tensor_sub(
                        smx[:, :, e], logits_sb[:, :, e], m1[:]
                    )
                nc.scalar.activation(
                    smx[:].rearrange("p a b -> p (a b)"),
                    smx[:].rearrange("p a b -> p (a b)"), ACT.Exp,
                )
                zsum = gsoft_pool.tile([128, NT], F32)
                nc.vector.tensor_reduce(zsum[:], smx[:], AXX.X, ALU.add)
                rz = gsoft_pool.tile([128, NT], F32)
                nc.vector.reciprocal(rz[:], zsum[:])
                gt8 = gsoft_pool.tile([128, NT, E], F32)
                for e in range(E):
                    nc.vector.tensor_tensor(
                        gt8[:, :, e], logits_sb[:, :, e], logits_sb[:, :, 0],
                        op=ALU.is_gt,
                    )
                cnt = gsoft_pool.tile([128, NT], F32)
                nc.vector.tensor_reduce(cnt[:], gt8[:], AXX.X, ALU.add)
                mask = gsoft_pool.tile([128, NT], F32)
                nc.vector.tensor_scalar(mask[:], cnt[:], 1.5, None, op0=ALU.is_lt)
                mm1 = gsoft_pool.tile([128, NT], F32)
                nc.vector.tensor_scalar_add(mm1[:], mask[:], -1.0)
                cw0 = gsoft_pool.tile([128, NT], F32)
                nc.vector.tensor_tensor(cw0[:], smx[:, :, 0], rz[:], op=ALU.mult)
                cand_id = gsoft_pool.tile([128, NT], F32)
                cand_cw = gsoft_pool.tile([128, NT], F32)
                nc.vector.tensor_tensor(cand_cw[:], cw0[:], mask[:], op=ALU.mult)
                nc.vector.tensor_add(cand_cw[:], cand_cw[:], mm1[:])
                nc.vector.tensor_tensor(cand_id[:], ids_sb[:], mask[:], op=ALU.mult)
                nc.vector.tensor_add(cand_id[:], cand_id[:], mm1[:])

                # ---- compaction ----
                # regroup [128,32] (p=t%128, i=t//128) -> [16,256] (t%16,t//16)
                scr_id = dram_pool.tile([T], F32, tag="scr_id")
                scr_cw = dram_pool.tile([T], F32, tag="scr_cw")
                nc.gpsimd.dma_start(
                    scr_id[:].rearrange("(i p) -> p i", p=128), cand_id[:]
                )
                nc.gpsimd.dma_start(
                    scr_cw[:].rearrange("(i p) -> p i", p=128), cand_cw[:]
                )
                cand16_id = gsoft_pool.tile([16, CAND_F], F32)
                cand16_cw = gsoft_pool.tile([16, CAND_F], F32)
                nc.gpsimd.dma_start(
                    cand16_id[:, 0:SENT_F],
                    scr_id[:].rearrange("(f b) -> b f", b=16),
                )
                nc.gpsimd.dma_start(
                    cand16_cw[:, 0:SENT_F],
                    scr_cw[:].rearrange("(f b) -> b f", b=16),
                )
                # sentinel candidates: token T (junk row), weight 0
                nc.any.memset(cand16_id[:, SENT_F:CAND_F], float(T))
                nc.any.memset(cand16_cw[:, SENT_F:CAND_F], 0.0)

                # output sized = input so compaction can never overflow; only
                # the first CAPF free-columns (1536 slots) are used downstream.
                sg_id = gsoft_pool.tile([16, CAND_F], F32)
                sg_cw = gsoft_pool.tile([16, CAND_F], F32)
                nf1 = gsoft_pool.tile([1, 1], U32)
                nf2 = gsoft_pool.tile([1, 1], U32)
                nc.gpsimd.sparse_gather(sg_id[:], cand16_id[:], num_found=nf1[:])
                nc.gpsimd.sparse_gather(sg_cw[:], cand16_cw[:], num_found=nf2[:])

                # int16 + replicate to all 8 16-partition groups
                nc.vector.tensor_copy(idx_rep[0:16, :], sg_id[:, 0:CAPF])
                nc.gpsimd.dma_start(idx_rep[16:32, :], idx_rep[0:16, :])
                nc.gpsimd.dma_start(idx_rep[32:64, :], idx_rep[0:32, :])
                nc.gpsimd.dma_start(idx_rep[64:128, :], idx_rep[0:64, :])

                # combine weights (s%16, s//16) -> slot-major [128, 12]
                scr_cw2 = dram_pool.tile([CAP], F32, tag="scr_cw2")
                nc.gpsimd.dma_start(
                    scr_cw2[:].rearrange("(f b) -> b f", b=16), sg_cw[:, 0:CAPF]
                )
                nc.gpsimd.dma_start(
                    cw_sl[:], scr_cw2[:].rearrange("(j p) -> p j", p=128)
                )

            # =========== FFN PHASE ===========
            with (
                tc.tile_pool(name="fxg", bufs=2) as fxg_pool,
                tc.tile_pool(name="fxt", bufs=2) as fxt_pool,
                tc.tile_pool(name="fh", bufs=1) as fh_pool,
                tc.tile_pool(name="fwd", bufs=6) as fwd_pool,
                tc.tile_pool(name="fy", bufs=1) as fy_pool,
                tc.tile_pool(name="fps_tr", bufs=2, space="PSUM") as fps_tr,
                tc.tile_pool(name="fps_up", bufs=2, space="PSUM") as fps_up,
                tc.tile_pool(name="fps_dn", bufs=4, space="PSUM") as fps_dn,
            ):
                for c in range(NCH):
                    idx_c = idx_rep[:, c * (CHS // 16):(c + 1) * (CHS // 16)]
                    xg = fxg_pool.tile([128, CHS // 128, D], F32, tag="xg")
                    nc.gpsimd.dma_gather(
                        xg[:], x_pad[:], idx_c, num_idxs=CHS,
                        num_idxs_reg=CHS, elem_size=D,
                    )
                    xcT = fxt_pool.tile([128, DC, CHS], F32R, tag="xcT")
                    for j in range(CHS // 128):
                        for kc in range(DC):
                            pt = fps_tr.tile([128, 128], F32, tag="tr")
                            nc.tensor.matmul(
                                pt[:], xg[:, j, ts(kc, 128)], ident_sb[:]
                            )
                            nc.any.tensor_copy(
                                xcT[:, kc, ds(j * 128, 128)], pt[:]
                            )
                    # up-projection + gelu -> h^T [128, FC, CHS]
                    h_sb = fh_pool.tile([128, FC, CHS], F32R, tag="h")
                    for m in range(FC):
                        psu = fps_up.tile([128, CHS], F32, tag="up")
                        for kc in range(DC):
                            nc.tensor.matmul(
                                psu[:],
                                wup_sb[:, kc, ts(m, 128)],
                                xcT[:, kc, :],
                                start=(kc == 0), stop=(kc == DC - 1),
                            )
                        nc.scalar.activation(
                            h_sb[:, m, :], psu[:], ACT.Gelu,
                            bias=bup_sb[:, m:m + 1],
                        )
                    # down-projection, natural orientation, + bias + scale
                    y_lo = fy_pool.tile([128, 2, D], F32, tag="ylo")
                    y_hi = fy_pool.tile([128, 2, D], F32, tag="yhi")
                    y_parts = [y_lo, y_hi]
                    for half in range(2):
                        psd = []
                        for _pi in range(CHS // 128):
                            psd_t = fps_dn.tile([128, HW_], F32, tag="dn")
                            psd.append(psd_t)
                        for m in range(FC):
                            wdn_mh = fwd_pool.tile([128, HW_], F32R, tag="wdn")
                            nc.scalar.dma_start(
                                wdn_mh[:], wdn_r[half * FC + m, :, :]
                            )
                            for blk in range(CHS // 128):
                                nc.tensor.matmul(
                                    psd[blk][:],
                                    h_sb[:, m, ts(blk, 128)],
                                    wdn_mh[:],
                                    start=(m == 0), stop=False,
                                )
                        for blk in range(CHS // 128):
                            nc.tensor.matmul(
                                psd[blk][:],
                                ones_sb[0:1, 0:128],
                                bdn_sb[0:1, ds(half * HW_, HW_)],
                                start=False, stop=True,
                            )
                            nc.vector.tensor_scalar(
                                y_parts[blk // 2][:, blk % 2, ds(half * HW_, HW_)],
                                psd[blk][:],
                                cw_sl[:, c * (CHS // 128) + blk:
                                      c * (CHS // 128) + blk + 1],
                                None,
                                op0=ALU.mult,
                            )
                    for yp in range(2):
                        nc.gpsimd.dma_scatter_add(
                            out_pad[:], y_parts[yp][:],
                            idx_rep[:, (c * 32 + yp * 16):(c * 32 + yp * 16 + 16)],
                            num_idxs=CHS // 2,
                            num_idxs_reg=CHS // 2, elem_size=D,
                        )

    nc.finalize()
    return nc


_NC_CACHE = None


def _get_program():
    global _NC_CACHE
    if _NC_CACHE is None:
        _NC_CACHE = build_program()
    return _NC_CACHE


def make_in_maps(hidden_states, gate_w, w_up, b_up, w_down, b_down):
    hidden_states = np.asarray(hidden_states, dtype=np.float32)
    gate_w = np.asarray(gate_w, dtype=np.float32)
    w_up = np.asarray(w_up, dtype=np.float32)
    b_up = np.asarray(b_up, dtype=np.float32)
    w_down = np.asarray(w_down, dtype=np.float32)
    b_down = np.asarray(b_down, dtype=np.float32)

    x = hidden_states.reshape(T, D)
    x_pad = np.zeros((TPAD, D), dtype=np.float32)
    x_pad[:T] = x
    ids = np.arange(T, dtype=np.float32).reshape(NT, 128).T.copy()  # [128, NT]
    ident = np.eye(128, dtype=np.float32)

    in_maps = []
    for c in range(E):
        gwc = np.concatenate([gate_w[:, c:], gate_w[:, :c]], axis=1).copy()
        wdn = w_down[c]  # [F, D]
        wdn_r = np.ascontiguousarray(
            wdn.reshape(FC, 128, 2, HW_).transpose(2, 0, 1, 3)
        ).reshape(2 * FC, 128, HW_)
        in_maps.append({
            "x_pad": x_pad,
            "gwc": gwc,
            "wup": np.ascontiguousarray(w_up[c]),
            "bup": np.ascontiguousarray(b_up[c]),
            "wdn_r": wdn_r,
            "bdn": np.ascontiguousarray(b_down[c]),
            "ids": ids,
            "ident": ident,
            "ones": np.ones((1, 128), dtype=np.float32),
        })
    return in_maps


def combine_results(results):
    out = np.zeros((T, D), dtype=np.float32)
    for c in range(E):
        out += results[c]["out_pad"][:T]
    return out.reshape(B, S, D)


def kernel(hidden_states, gate_w, w_up, b_up, w_down, b_down):
    in_maps = make_in_maps(hidden_states, gate_w, w_up, b_up, w_down, b_down)
    nc = _get_program()
    res = run_bass_kernel_spmd(nc, in_maps, core_ids=list(range(E)))
    return combine_results(res.results)


if __name__ == "__main__":
    rng = np.random.default_rng(0)
    hs = rng.standard_normal((B, S, D)).astype(np.float32)
    gw = rng.standard_normal((D, E)).astype(np.float32) / np.sqrt(D)
    wu = (rng.standard_normal((E, D, F)) * 0.02).astype(np.float32)
    bu = np.zeros((E, F), dtype=np.float32)
    wd = (rng.standard_normal((E, F, D)) * 0.02).astype(np.float32)
    bd = np.zeros((E, D), dtype=np.float32)
    out = kernel(hs, gw, wu, bu, wd, bd)
    print("out", out.shape, out.dtype, np.abs(out).max())


# revision 15
# speedup vs baseline: 1.2288x; 1.2227x over previous
"""Trainium2 Bass kernel for nn_BertMoELayer (B=2,S=2048,D=768,F=3072,E=8,top-2).

Strategy: expert-parallel across 8 NeuronCores (1 expert per core).
Each core receives the full token set, computes the router (fp32), selects
the tokens routed to its expert (top-2 membership), compacts their indices
on-device (sparse_gather), gathers the token rows (dma_gather), runs the
expert FFN in fp32r (PE fast mode), scales by the combine weight, and
scatter-adds the rows into a zeroed output buffer (dma_scatter_add).
Host sums the 8 partial outputs.

Self-contained: hardcodes all shapes; only imports the installed concourse
stack from /opt/trn_rl_repo.
"""
import sys

sys.path.insert(0, "/opt/trn_rl_repo")

import numpy as np

import concourse.bass as bass
import concourse.tile as tile
from concourse import bacc, mybir
from concourse.bass import ds, ts
from concourse.bass_utils import run_bass_kernel_spmd

# Problem shapes
B, S, D, F, E = 2, 2048, 768, 3072, 8
T = B * S                 # 4096 tokens
CAP = 1536                # per-expert slot capacity (expected load 1024)
TPAD = T + CAP            # token rows incl. junk region for sentinel slots
DC = D // 128             # 6 contraction chunks for up-proj
FC = F // 128             # 24 contraction chunks for down-proj
NT = T // 128             # 32 token tiles
NCH = 3                   # FFN slot chunks
CHS = CAP // NCH          # 512 slots per chunk
NG = 8                    # gate groups of 512 tokens
CAND_F = (T + CAP) // 16  # 352 candidate free-dim
SENT_F = T // 16          # 256: sentinel region starts here
CAPF = CAP // 16          # 96
HW_ = D // 2              # 384: down-proj half width

F32 = mybir.dt.float32
F32R = mybir.dt.float32r
I16 = mybir.dt.int16
U32 = mybir.dt.uint32
ALU = mybir.AluOpType
AXX = mybir.AxisListType
ACT = mybir.ActivationFunctionType


def build_program():
    nc = bacc.Bacc("TRN2", target_bir_lowering=False, debug=False)

    x_pad = nc.dram_tensor("x_pad", (TPAD, D), F32, kind="ExternalInput")
    gwc = nc.dram_tensor("gwc", (D, E), F32, kind="ExternalInput")
    wup = nc.dram_tensor("wup", (D, F), F32R, kind="ExternalInput")
    bup = nc.dram_tensor("bup", (F,), F32, kind="ExternalInput")
    # wdn_r[half*FC + m] = w_down[m*128:(m+1)*128, half*384:(half+1)*384]
    wdn_r = nc.dram_tensor("wdn_r", (2 * FC, 128, HW_), F32R,
                           kind="ExternalInput")
    bdn = nc.dram_tensor("bdn", (D,), F32R, kind="ExternalInput")
    ids = nc.dram_tensor("ids", (128, NT), F32, kind="ExternalInput")
    ident = nc.dram_tensor("ident", (128, 128), F32, kind="ExternalInput")
    ones = nc.dram_tensor("ones", (1, 128), F32R, kind="ExternalInput")
    out_pad = nc.dram_tensor("out_pad", (TPAD, D), F32, kind="ExternalOutput")

    with tile.TileContext(nc) as tc:
        with (
            tc.tile_pool(name="const", bufs=1) as const_pool,
            tc.tile_pool(name="dram", bufs=1, space="DRAM") as dram_pool,
            tc.tile_pool(name="route", bufs=1) as route_pool,
        ):
            # ---- constants / small inputs ----
            ident_sb = const_pool.tile([128, 128], F32)
            nc.sync.dma_start(ident_sb[:], ident[:])
            gwc_sb = const_pool.tile([128, DC, E], F32)
            nc.sync.dma_start(gwc_sb[:], gwc.rearrange("(kc p) e -> p kc e", p=128))
            ids_sb = const_pool.tile([128, NT], F32)
            nc.sync.dma_start(ids_sb[:], ids[:])
            bup_sb = const_pool.tile([128, FC], F32)
            nc.scalar.dma_start(bup_sb[:], bup.rearrange("(m p) -> p m", p=128))
            bdn_sb = const_pool.tile([1, D], F32R)
            nc.scalar.dma_start(bdn_sb[:], bdn[None, :])
            ones_sb = const_pool.tile([1, 128], F32R)
            nc.scalar.dma_start(ones_sb[:], ones[:])

            # ---- resident up-proj weights (scalar rail; needed only at FFN) ----
            wup_sb = const_pool.tile([128, DC, F], F32R)
            nc.scalar.dma_start(wup_sb[:], wup.rearrange("(kc p) f -> p kc f", p=128))

            # ---- routing products (survive into the FFN phase) ----
            idx_rep = route_pool.tile([128, CAPF], I16)
            cw_sl = route_pool.tile([128, CAP // 128], F32)

            # =========== GATE PHASE ===========
            with (
                tc.tile_pool(name="gzero", bufs=1) as gz_pool,
                tc.tile_pool(name="gin", bufs=2) as gin_pool,
                tc.tile_pool(name="gxt", bufs=2) as gxt_pool,
                tc.tile_pool(name="glt", bufs=2) as glt_pool,
                tc.tile_pool(name="gsoft", bufs=1) as gsoft_pool,
                tc.tile_pool(name="gps_tr", bufs=2, space="PSUM") as gps_tr,
                tc.tile_pool(name="gps_lt", bufs=2, space="PSUM") as gps_lt,
                tc.tile_pool(name="gps_ln", bufs=2, space="PSUM") as gps_ln,
            ):
                # zero the real-token region of out_pad
                zt = gz_pool.tile([128, 4, D], F32)
                nc.any.memset(zt[:], 0.0)
                out_zv = out_pad[0:T, :].rearrange("(n p) d -> p n d", p=128)
                for z in range(NT // 4):
                    nc.scalar.dma_start(out_zv[:, ts(z, 4), :], zt[:])

                logits_sb = gsoft_pool.tile([128, NT, E], F32)
                for g in range(NG):
                    xg_nat = gin_pool.tile([128, 4, D], F32, tag="xg")
                    nc.sync.dma_start(
                        xg_nat[:],
                        x_pad[g * 512:(g + 1) * 512, :].rearrange(
                            "(j p) d -> p j d", p=128
                        ),
                    )
                    xT_g = gxt_pool.tile([128, DC, 512], F32, tag="xT")
                    for j in range(4):
                        for kc in range(DC):
                            pt = gps_tr.tile([128, 128], F32, tag="tr")
                            nc.tensor.matmul(
                                pt[:], xg_nat[:, j, ts(kc, 128)], ident_sb[:]
                            )
                            nc.any.tensor_copy(
                                xT_g[:, kc, ds(j * 128, 128)], pt[:]
                            )
                    lps = gps_lt.tile([8, 512], F32, tag="lt")
                    for kc in range(DC):
                        nc.tensor.matmul(
                            lps[:], gwc_sb[:, kc, :], xT_g[:, kc, :],
                            start=(kc == 0), stop=(kc == DC - 1),
                        )
                    lT_sb = glt_pool.tile([8, 512], F32, tag="lT")
                    nc.any.tensor_copy(lT_sb[:], lps[:])
                    for j in range(4):
                        t = g * 4 + j
                        pn = gps_ln.tile([128, 8], F32, tag="ln")
                        nc.tensor.matmul(
                            pn[:], lT_sb[:, ts(j, 128)], ident_sb[0:8, 0:8]
                        )
                        nc.any.tensor_copy(logits_sb[:, t, :], pn[:])

                # ---- batched softmax + top-2 over all 32 tiles ----
                m1 = gsoft_pool.tile([128, NT], F32)
                nc.vector.tensor_reduce(m1[:], logits_sb[:], AXX.X, ALU.max)
                smx = gsoft_pool.tile([128, NT, E], F32)
                for e in range(E):
                    nc.vector.tensor_sub(
                        smx[:, :, e], logits_sb[:, :, e], m1[:]
                    )
                nc.scalar.activation(
                    smx[:].rearrange("p a b -> p (a b)"),
                    smx[:].rearrange("p a b -> p (a b)"), ACT.Exp,
                )
                zsum = gsoft_pool.tile([128, NT], F32)
                nc.vector.tensor_reduce(zsum[:], smx[:], AXX.X, ALU.add)
                rz = gsoft_pool.tile([128, NT], F32)
                nc.vector.reciprocal(rz[:], zsum[:])
                gt8 = gsoft_pool.tile([128, NT, E], F32)
                for e in range(E):
                    nc.vector.tensor_tensor(
                        gt8[:, :, e], logits_sb[:, :, e], logits_sb[:, :, 0],
                        op=ALU.is_gt,
                    )
                cnt = gsoft_pool.tile([128, NT], F32)
                nc.vector.tensor_reduce(cnt[:], gt8[:], AXX.X, ALU.add)
                mask = gsoft_pool.tile([128, NT], F32)
                nc.vector.tensor_scalar(mask[:], cnt[:], 1.5, None, op0=ALU.is_lt)
                mm1 = gsoft_pool.tile([128, NT], F32)
                nc.vector.tensor_scalar_add(mm1[:], mask[:], -1.0)
                cw0 = gsoft_pool.tile([128, NT], F32)
                nc.vector.tensor_tensor(cw0[:], smx[:, :, 0], rz[:], op=ALU.mult)
                cand_id = gsoft_pool.tile([128, NT], F32)
                cand_cw = gsoft_pool.tile([128, NT], F32)
                nc.vector.tensor_tensor(cand_cw[:], cw0[:], mask[:], op=ALU.mult)
                nc.vector.tensor_add(cand_cw[:], cand_cw[:], mm1[:])
                nc.vector.tensor_tensor(cand_id[:], ids_sb[:], mask[:], op=ALU.mult)
                nc.vector.tensor_add(cand_id[:], cand_id[:], mm1[:])

                # ---- compaction ----
                # regroup [128,32] -> [16,256] via PE transpose (any candidate
                # order works; only "sentinels last" matters)
                cand16_id = gsoft_pool.tile([16, CAND_F], F32)
                cand16_cw = gsoft_pool.tile([16, CAND_F], F32)
                for cbuf, c16 in ((cand_id, cand16_id), (cand_cw, cand16_cw)):
                    pct = gps_tr.tile([32, 128], F32, tag="tr")
                    nc.tensor.matmul(pct[:], cbuf[:], ident_sb[:])
                    ctT = gsoft_pool.tile([32, 128], F32, tag="ctT")
                    nc.any.tensor_copy(ctT[:], pct[:])
                    nc.vector.tensor_copy(c16[:, 0:128], ctT[0:16, :])
                    nc.gpsimd.dma_start(c16[:, 128:256], ctT[16:32, :])
                # sentinel candidates: token T (junk row), weight 0
                nc.any.memset(cand16_id[:, SENT_F:CAND_F], float(T))
                nc.any.memset(cand16_cw[:, SENT_F:CAND_F], 0.0)

                # output sized = input so compaction can never overflow; only
                # the first CAPF free-columns (1536 slots) are used downstream.
                sg_id = gsoft_pool.tile([16, CAND_F], F32)
                sg_cw = gsoft_pool.tile([16, CAND_F], F32)
                nf1 = gsoft_pool.tile([1, 1], U32)
                nf2 = gsoft_pool.tile([1, 1], U32)
                nc.gpsimd.sparse_gather(sg_id[:], cand16_id[:], num_found=nf1[:])
                nc.gpsimd.sparse_gather(sg_cw[:], cand16_cw[:], num_found=nf2[:])

                # int16 + replicate to all 8 16-partition groups
                nc.vector.tensor_copy(idx_rep[0:16, :], sg_id[:, 0:CAPF])
                nc.gpsimd.dma_start(idx_rep[16:32, :], idx_rep[0:16, :])
                nc.gpsimd.dma_start(idx_rep[32:64, :], idx_rep[0:32, :])
                nc.gpsimd.dma_start(idx_rep[64:128, :], idx_rep[0:64, :])

                # combine weights (s%16, s//16) -> slot-major [128, 12]:
                # [16,96] -T-> [96,16] -> DRAM slot-order -> [12,128] -T-> [128,12]
                pcw = gps_tr.tile([96, 16], F32, tag="tr")
                nc.tensor.matmul(pcw[:], sg_cw[:, 0:CAPF], ident_sb[0:16, 0:16])
                cwT = gsoft_pool.tile([96, 16], F32)
                nc.any.tensor_copy(cwT[:], pcw[:])
                scr_cw2 = dram_pool.tile([CAP], F32, tag="scr_cw2")
                nc.gpsimd.dma_start(
                    scr_cw2[:].rearrange("(f b) -> f b", b=16), cwT[:]
                )
                cw12 = gsoft_pool.tile([12, 128], F32)
                nc.gpsimd.dma_start(
                    cw12[:], scr_cw2[:].rearrange("(j p) -> j p", p=128)
                )
                pcw2 = gps_tr.tile([128, 12], F32, tag="tr")
                nc.tensor.matmul(pcw2[:], cw12[:], ident_sb[0:12, 0:12])
                nc.any.tensor_copy(cw_sl[:], pcw2[:])

            # =========== FFN PHASE ===========
            with (
                tc.tile_pool(name="fxg", bufs=2) as fxg_pool,
                tc.tile_pool(name="fxt", bufs=2) as fxt_pool,
                tc.tile_pool(name="fh", bufs=1) as fh_pool,
                tc.tile_pool(name="fwd", bufs=6) as fwd_pool,
                tc.tile_pool(name="fy", bufs=1) as fy_pool,
                tc.tile_pool(name="fps_tr", bufs=2, space="PSUM") as fps_tr,
                tc.tile_pool(name="fps_up", bufs=2, space="PSUM") as fps_up,
                tc.tile_pool(name="fps_dn", bufs=4, space="PSUM") as fps_dn,
            ):
                for c in range(NCH):
                    idx_c = idx_rep[:, c * (CHS // 16):(c + 1) * (CHS // 16)]
                    xg = fxg_pool.tile([128, CHS // 128, D], F32, tag="xg")
                    nc.gpsimd.dma_gather(
                        xg[:], x_pad[:], idx_c, num_idxs=CHS,
                        num_idxs_reg=CHS, elem_size=D,
                    )
                    xcT = fxt_pool.tile([128, DC, CHS], F32R, tag="xcT")
                    for j in range(CHS // 128):
                        for kc in range(DC):
                            pt = fps_tr.tile([128, 128], F32, tag="tr")
                            nc.tensor.matmul(
                                pt[:], xg[:, j, ts(kc, 128)], ident_sb[:]
                            )
                            nc.any.tensor_copy(
                                xcT[:, kc, ds(j * 128, 128)], pt[:]
                            )
                    # up-projection + gelu -> h^T [128, FC, CHS]
                    h_sb = fh_pool.tile([128, FC, CHS], F32R, tag="h")
                    for m in range(FC):
                        psu = fps_up.tile([128, CHS], F32, tag="up")
                        for kc in range(DC):
                            nc.tensor.matmul(
                                psu[:],
                                wup_sb[:, kc, ts(m, 128)],
                                xcT[:, kc, :],
                                start=(kc == 0), stop=(kc == DC - 1),
                            )
                        nc.scalar.activation(
                            h_sb[:, m, :], psu[:], ACT.Gelu,
                            bias=bup_sb[:, m:m + 1],
                        )
                    # down-projection, natural orientation, + bias + scale
                    y_lo = fy_pool.tile([128, 2, D], F32, tag="ylo")
                    y_hi = fy_pool.tile([128, 2, D], F32, tag="yhi")
                    y_parts = [y_lo, y_hi]
                    for half in range(2):
                        psd = []
                        for _pi in range(CHS // 128):
                            psd_t = fps_dn.tile([128, HW_], F32, tag="dn")
                            psd.append(psd_t)
                        for m2 in range(FC // 2):
                            wdn_mh = fwd_pool.tile([128, 2, HW_], F32R, tag="wdn")
                            nc.scalar.dma_start(
                                wdn_mh[:],
                                wdn_r[half * FC + 2 * m2:half * FC + 2 * m2 + 2,
                                      :, :].rearrange("two p h -> p two h"),
                            )
                            for mi in range(2):
                                m = 2 * m2 + mi
                                for blk in range(CHS // 128):
                                    nc.tensor.matmul(
                                        psd[blk][:],
                                        h_sb[:, m, ts(blk, 128)],
                                        wdn_mh[:, mi, :],
                                        start=(m == 0), stop=False,
                                    )
                        for blk in range(CHS // 128):
                            nc.tensor.matmul(
                                psd[blk][:],
                                ones_sb[0:1, 0:128],
                                bdn_sb[0:1, ds(half * HW_, HW_)],
                                start=False, stop=True,
                            )
                            nc.vector.tensor_scalar(
                                y_parts[blk // 2][:, blk % 2, ds(half * HW_, HW_)],
                                psd[blk][:],
                                cw_sl[:, c * (CHS // 128) + blk:
                                      c * (CHS // 128) + blk + 1],
                                None,
                                op0=ALU.mult,
                            )
                    for yp in range(2):
                        nc.gpsimd.dma_scatter_add(
                            out_pad[:], y_parts[yp][:],
                            idx_rep[:, (c * 32 + yp * 16):(c * 32 + yp * 16 + 16)],
                            num_idxs=CHS // 2,
                            num_idxs_reg=CHS // 2, elem_size=D,
                        )

    nc.finalize()
    return nc


_NC_CACHE = None


def _get_program():
    global _NC_CACHE
    if _NC_CACHE is None:
        _NC_CACHE = build_program()
    return _NC_CACHE


def make_in_maps(hidden_states, gate_w, w_up, b_up, w_down, b_down):
    hidden_states = np.asarray(hidden_states, dtype=np.float32)
    gate_w = np.asarray(gate_w, dtype=np.float32)
    w_up = np.asarray(w_up, dtype=np.float32)
    b_up = np.asarray(b_up, dtype=np.float32)
    w_down = np.asarray(w_down, dtype=np.float32)
    b_down = np.asarray(b_down, dtype=np.float32)

    x = hidden_states.reshape(T, D)
    x_pad = np.zeros((TPAD, D), dtype=np.float32)
    x_pad[:T] = x
    ids = np.arange(T, dtype=np.float32).reshape(NT, 128).T.copy()  # [128, NT]
    ident = np.eye(128, dtype=np.float32)

    in_maps = []
    for c in range(E):
        gwc = np.concatenate([gate_w[:, c:], gate_w[:, :c]], axis=1).copy()
        wdn = w_down[c]  # [F, D]
        wdn_r = np.ascontiguousarray(
            wdn.reshape(FC, 128, 2, HW_).transpose(2, 0, 1, 3)
        ).reshape(2 * FC, 128, HW_)
        in_maps.append({
            "x_pad": x_pad,
            "gwc": gwc,
            "wup": np.ascontiguousarray(w_up[c]),
            "bup": np.ascontiguousarray(b_up[c]),
            "wdn_r": wdn_r,
            "bdn": np.ascontiguousarray(b_down[c]),
            "ids": ids,
            "ident": ident,
            "ones": np.ones((1, 128), dtype=np.float32),
        })
    return in_maps


def combine_results(results):
    out = np.zeros((T, D), dtype=np.float32)
    for c in range(E):
        out += results[c]["out_pad"][:T]
    return out.reshape(B, S, D)


def kernel(hidden_states, gate_w, w_up, b_up, w_down, b_down):
    in_maps = make_in_maps(hidden_states, gate_w, w_up, b_up, w_down, b_down)
    nc = _get_program()
    res = run_bass_kernel_spmd(nc, in_maps, core_ids=list(range(E)))
    return combine_results(res.results)


if __name__ == "__main__":
    rng = np.random.default_rng(0)
    hs = rng.standard_normal((B, S, D)).astype(np.float32)
    gw = rng.standard_normal((D, E)).astype(np.float32) / np.sqrt(D)
    wu = (rng.standard_normal((E, D, F)) * 0.02).astype(np.float32)
    bu = np.zeros((E, F), dtype=np.float32)
    wd = (rng.standard_normal((E, F, D)) * 0.02).astype(np.float32)
    bd = np.zeros((E, D), dtype=np.float32)
    out = kernel(hs, gw, wu, bu, wd, bd)
    print("out", out.shape, out.dtype, np.abs(out).max())


# revision 16
# speedup vs baseline: 1.3661x; 1.1117x over previous
"""Trainium2 Bass kernel for nn_BertMoELayer (B=2,S=2048,D=768,F=3072,E=8,top-2).

Strategy: expert-parallel across 8 NeuronCores (1 expert per core).
Each core receives the full token set, computes the router (fp32), selects
the tokens routed to its expert (top-2 membership), compacts their indices
on-device (sparse_gather), gathers the token rows (dma_gather), runs the
expert FFN in fp32r (PE fast mode), scales by the combine weight, and
scatter-adds the rows into a zeroed output buffer (dma_scatter_add).
Host sums the 8 partial outputs.

Self-contained: hardcodes all shapes; only imports the installed concourse
stack from /opt/trn_rl_repo.
"""
import sys

sys.path.insert(0, "/opt/trn_rl_repo")

import numpy as np

import concourse.bass as bass
import concourse.tile as tile
from concourse import bacc, mybir
from concourse.bass import ds, ts
from concourse.bass_utils import run_bass_kernel_spmd

# Problem shapes
B, S, D, F, E = 2, 2048, 768, 3072, 8
T = B * S                 # 4096 tokens
CAP = 1536                # per-expert slot capacity (expected load 1024)
TPAD = T + CAP            # token rows incl. junk region for sentinel slots
DC = D // 128             # 6 contraction chunks for up-proj
FC = F // 128             # 24 contraction chunks for down-proj
NT = T // 128             # 32 token tiles
NCH = 3                   # FFN slot chunks
CHS = CAP // NCH          # 512 slots per chunk
NG = 8                    # gate groups of 512 tokens
CAND_F = (T + CAP) // 16  # 352 candidate free-dim
SENT_F = T // 16          # 256: sentinel region starts here
CAPF = CAP // 16          # 96
HW_ = D // 2              # 384: down-proj half width

F32 = mybir.dt.float32
F32R = mybir.dt.float32r
I16 = mybir.dt.int16
U32 = mybir.dt.uint32
ALU = mybir.AluOpType
AXX = mybir.AxisListType
ACT = mybir.ActivationFunctionType


def build_program():
    nc = bacc.Bacc("TRN2", target_bir_lowering=False, debug=False)

    x_pad = nc.dram_tensor("x_pad", (TPAD, D), F32, kind="ExternalInput")
    xt = nc.dram_tensor("xt", (D, T), F32, kind="ExternalInput")
    gwc = nc.dram_tensor("gwc", (D, E), F32, kind="ExternalInput")
    wup = nc.dram_tensor("wup", (D, F), F32R, kind="ExternalInput")
    bup = nc.dram_tensor("bup", (F,), F32, kind="ExternalInput")
    # wdn_r[half*FC + m] = w_down[m*128:(m+1)*128, half*384:(half+1)*384]
    wdn_r = nc.dram_tensor("wdn_r", (2 * FC, 128, HW_), F32R,
                           kind="ExternalInput")
    bdn = nc.dram_tensor("bdn", (D,), F32R, kind="ExternalInput")
    ids = nc.dram_tensor("ids", (128, NT), F32, kind="ExternalInput")
    ident = nc.dram_tensor("ident", (128, 128), F32, kind="ExternalInput")
    ones = nc.dram_tensor("ones", (1, 128), F32R, kind="ExternalInput")
    out_pad = nc.dram_tensor("out_pad", (TPAD, D), F32, kind="ExternalOutput")

    with tile.TileContext(nc) as tc:
        with (
            tc.tile_pool(name="const", bufs=1) as const_pool,
            tc.tile_pool(name="dram", bufs=1, space="DRAM") as dram_pool,
            tc.tile_pool(name="route", bufs=1) as route_pool,
        ):
            # ---- constants / small inputs ----
            ident_sb = const_pool.tile([128, 128], F32)
            nc.sync.dma_start(ident_sb[:], ident[:])
            gwc_sb = const_pool.tile([128, DC, E], F32)
            nc.sync.dma_start(gwc_sb[:], gwc.rearrange("(kc p) e -> p kc e", p=128))
            ids_sb = const_pool.tile([128, NT], F32)
            nc.sync.dma_start(ids_sb[:], ids[:])
            bup_sb = const_pool.tile([128, FC], F32)
            nc.scalar.dma_start(bup_sb[:], bup.rearrange("(m p) -> p m", p=128))
            bdn_sb = const_pool.tile([1, D], F32R)
            nc.scalar.dma_start(bdn_sb[:], bdn[None, :])
            ones_sb = const_pool.tile([1, 128], F32R)
            nc.scalar.dma_start(ones_sb[:], ones[:])

            # ---- resident up-proj weights (scalar rail; needed only at FFN) ----
            wup_sb = const_pool.tile([128, DC, F], F32R)
            nc.scalar.dma_start(wup_sb[:], wup.rearrange("(kc p) f -> p kc f", p=128))

            # ---- routing products (survive into the FFN phase) ----
            idx_rep = route_pool.tile([128, CAPF], I16)
            cw_sl = route_pool.tile([128, CAP // 128], F32)

            # =========== GATE PHASE ===========
            with (
                tc.tile_pool(name="gzero", bufs=1) as gz_pool,
                tc.tile_pool(name="gxt", bufs=2) as gxt_pool,
                tc.tile_pool(name="glt", bufs=2) as glt_pool,
                tc.tile_pool(name="gsoft", bufs=1) as gsoft_pool,
                tc.tile_pool(name="gps_tr", bufs=2, space="PSUM") as gps_tr,
                tc.tile_pool(name="gps_lt", bufs=2, space="PSUM") as gps_lt,
                tc.tile_pool(name="gps_ln", bufs=2, space="PSUM") as gps_ln,
            ):
                # zero the real-token region of out_pad
                zt = gz_pool.tile([128, 4, D], F32)
                nc.any.memset(zt[:], 0.0)
                out_zv = out_pad[0:T, :].rearrange("(n p) d -> p n d", p=128)
                for z in range(NT // 4):
                    nc.scalar.dma_start(out_zv[:, ts(z, 4), :], zt[:])

                logits_sb = gsoft_pool.tile([128, NT, E], F32)
                for g in range(NG):
                    xT_g = gxt_pool.tile([128, DC, 512], F32, tag="xT")
                    nc.sync.dma_start(
                        xT_g[:],
                        xt[:, g * 512:(g + 1) * 512].rearrange(
                            "(kc p) t -> p kc t", p=128
                        ),
                    )
                    lps = gps_lt.tile([8, 512], F32, tag="lt")
                    for kc in range(DC):
                        nc.tensor.matmul(
                            lps[:], gwc_sb[:, kc, :], xT_g[:, kc, :],
                            start=(kc == 0), stop=(kc == DC - 1),
                        )
                    lT_sb = glt_pool.tile([8, 512], F32, tag="lT")
                    nc.any.tensor_copy(lT_sb[:], lps[:])
                    for j in range(4):
                        t = g * 4 + j
                        pn = gps_ln.tile([128, 8], F32, tag="ln")
                        nc.tensor.matmul(
                            pn[:], lT_sb[:, ts(j, 128)], ident_sb[0:8, 0:8]
                        )
                        nc.any.tensor_copy(logits_sb[:, t, :], pn[:])

                # ---- batched softmax + top-2 over all 32 tiles ----
                m1 = gsoft_pool.tile([128, NT], F32)
                nc.vector.tensor_reduce(m1[:], logits_sb[:], AXX.X, ALU.max)
                smx = gsoft_pool.tile([128, NT, E], F32)
                for e in range(E):
                    nc.vector.tensor_sub(
                        smx[:, :, e], logits_sb[:, :, e], m1[:]
                    )
                nc.scalar.activation(
                    smx[:].rearrange("p a b -> p (a b)"),
                    smx[:].rearrange("p a b -> p (a b)"), ACT.Exp,
                )
                zsum = gsoft_pool.tile([128, NT], F32)
                nc.vector.tensor_reduce(zsum[:], smx[:], AXX.X, ALU.add)
                rz = gsoft_pool.tile([128, NT], F32)
                nc.vector.reciprocal(rz[:], zsum[:])
                gt8 = gsoft_pool.tile([128, NT, E], F32)
                for e in range(E):
                    nc.vector.tensor_tensor(
                        gt8[:, :, e], logits_sb[:, :, e], logits_sb[:, :, 0],
                        op=ALU.is_gt,
                    )
                cnt = gsoft_pool.tile([128, NT], F32)
                nc.vector.tensor_reduce(cnt[:], gt8[:], AXX.X, ALU.add)
                mask = gsoft_pool.tile([128, NT], F32)
                nc.vector.tensor_scalar(mask[:], cnt[:], 1.5, None, op0=ALU.is_lt)
                mm1 = gsoft_pool.tile([128, NT], F32)
                nc.vector.tensor_scalar_add(mm1[:], mask[:], -1.0)
                cw0 = gsoft_pool.tile([128, NT], F32)
                nc.vector.tensor_tensor(cw0[:], smx[:, :, 0], rz[:], op=ALU.mult)
                cand_id = gsoft_pool.tile([128, NT], F32)
                cand_cw = gsoft_pool.tile([128, NT], F32)
                nc.vector.tensor_tensor(cand_cw[:], cw0[:], mask[:], op=ALU.mult)
                nc.vector.tensor_add(cand_cw[:], cand_cw[:], mm1[:])
                nc.vector.tensor_tensor(cand_id[:], ids_sb[:], mask[:], op=ALU.mult)
                nc.vector.tensor_add(cand_id[:], cand_id[:], mm1[:])

                # ---- compaction ----
                # regroup [128,32] -> [16,256] via PE transpose (any candidate
                # order works; only "sentinels last" matters)
                cand16_id = gsoft_pool.tile([16, CAND_F], F32)
                cand16_cw = gsoft_pool.tile([16, CAND_F], F32)
                for cbuf, c16 in ((cand_id, cand16_id), (cand_cw, cand16_cw)):
                    pct = gps_tr.tile([32, 128], F32, tag="tr")
                    nc.tensor.matmul(pct[:], cbuf[:], ident_sb[:])
                    ctT = gsoft_pool.tile([32, 128], F32, tag="ctT")
                    nc.any.tensor_copy(ctT[:], pct[:])
                    nc.vector.tensor_copy(c16[:, 0:128], ctT[0:16, :])
                    nc.gpsimd.dma_start(c16[:, 128:256], ctT[16:32, :])
                # sentinel candidates: token T (junk row), weight 0
                nc.any.memset(cand16_id[:, SENT_F:CAND_F], float(T))
                nc.any.memset(cand16_cw[:, SENT_F:CAND_F], 0.0)

                # output sized = input so compaction can never overflow; only
                # the first CAPF free-columns (1536 slots) are used downstream.
                sg_id = gsoft_pool.tile([16, CAND_F], F32)
                sg_cw = gsoft_pool.tile([16, CAND_F], F32)
                nf1 = gsoft_pool.tile([1, 1], U32)
                nf2 = gsoft_pool.tile([1, 1], U32)
                nc.gpsimd.sparse_gather(sg_id[:], cand16_id[:], num_found=nf1[:])
                nc.gpsimd.sparse_gather(sg_cw[:], cand16_cw[:], num_found=nf2[:])

                # int16 + replicate to all 8 16-partition groups
                nc.vector.tensor_copy(idx_rep[0:16, :], sg_id[:, 0:CAPF])
                nc.gpsimd.dma_start(idx_rep[16:32, :], idx_rep[0:16, :])
                nc.gpsimd.dma_start(idx_rep[32:64, :], idx_rep[0:32, :])
                nc.gpsimd.dma_start(idx_rep[64:128, :], idx_rep[0:64, :])

                # combine weights (s%16, s//16) -> slot-major [128, 12]:
                # [16,96] -T-> [96,16] -> DRAM slot-order -> [12,128] -T-> [128,12]
                pcw = gps_tr.tile([96, 16], F32, tag="tr")
                nc.tensor.matmul(pcw[:], sg_cw[:, 0:CAPF], ident_sb[0:16, 0:16])
                cwT = gsoft_pool.tile([96, 16], F32)
                nc.any.tensor_copy(cwT[:], pcw[:])
                scr_cw2 = dram_pool.tile([CAP], F32, tag="scr_cw2")
                nc.gpsimd.dma_start(
                    scr_cw2[:].rearrange("(f b) -> f b", b=16), cwT[:]
                )
                cw12 = gsoft_pool.tile([12, 128], F32)
                nc.gpsimd.dma_start(
                    cw12[:], scr_cw2[:].rearrange("(j p) -> j p", p=128)
                )
                pcw2 = gps_tr.tile([128, 12], F32, tag="tr")
                nc.tensor.matmul(pcw2[:], cw12[:], ident_sb[0:12, 0:12])
                nc.any.tensor_copy(cw_sl[:], pcw2[:])

            # =========== FFN PHASE ===========
            with (
                tc.tile_pool(name="fxg", bufs=2) as fxg_pool,
                tc.tile_pool(name="fxt", bufs=2) as fxt_pool,
                tc.tile_pool(name="fh", bufs=1) as fh_pool,
                tc.tile_pool(name="fwd", bufs=6) as fwd_pool,
                tc.tile_pool(name="fy", bufs=1) as fy_pool,
                tc.tile_pool(name="fps_tr", bufs=2, space="PSUM") as fps_tr,
                tc.tile_pool(name="fps_up", bufs=2, space="PSUM") as fps_up,
                tc.tile_pool(name="fps_dn", bufs=4, space="PSUM") as fps_dn,
            ):
                for c in range(NCH):
                    idx_c = idx_rep[:, c * (CHS // 16):(c + 1) * (CHS // 16)]
                    xg = fxg_pool.tile([128, CHS // 128, D], F32, tag="xg")
                    nc.gpsimd.dma_gather(
                        xg[:], x_pad[:], idx_c, num_idxs=CHS,
                        num_idxs_reg=CHS, elem_size=D,
                    )
                    xcT = fxt_pool.tile([128, DC, CHS], F32R, tag="xcT")
                    for j in range(CHS // 128):
                        for kc in range(DC):
                            pt = fps_tr.tile([128, 128], F32, tag="tr")
                            nc.tensor.matmul(
                                pt[:], xg[:, j, ts(kc, 128)], ident_sb[:]
                            )
                            nc.any.tensor_copy(
                                xcT[:, kc, ds(j * 128, 128)], pt[:]
                            )
                    # up-projection + gelu -> h^T [128, FC, CHS]
                    h_sb = fh_pool.tile([128, FC, CHS], F32R, tag="h")
                    for m in range(FC):
                        psu = fps_up.tile([128, CHS], F32, tag="up")
                        for kc in range(DC):
                            nc.tensor.matmul(
                                psu[:],
                                wup_sb[:, kc, ts(m, 128)],
                                xcT[:, kc, :],
                                start=(kc == 0), stop=(kc == DC - 1),
                            )
                        nc.scalar.activation(
                            h_sb[:, m, :], psu[:], ACT.Gelu,
                            bias=bup_sb[:, m:m + 1],
                        )
                    # down-projection, natural orientation, + bias + scale
                    y_lo = fy_pool.tile([128, 2, D], F32, tag="ylo")
                    y_hi = fy_pool.tile([128, 2, D], F32, tag="yhi")
                    y_parts = [y_lo, y_hi]
                    for half in range(2):
                        psd = []
                        for _pi in range(CHS // 128):
                            psd_t = fps_dn.tile([128, HW_], F32, tag="dn")
                            psd.append(psd_t)
                        for m2 in range(FC // 2):
                            wdn_mh = fwd_pool.tile([128, 2, HW_], F32R, tag="wdn")
                            nc.scalar.dma_start(
                                wdn_mh[:],
                                wdn_r[half * FC + 2 * m2:half * FC + 2 * m2 + 2,
                                      :, :].rearrange("two p h -> p two h"),
                            )
                            for mi in range(2):
                                m = 2 * m2 + mi
                                for blk in range(CHS // 128):
                                    nc.tensor.matmul(
                                        psd[blk][:],
                                        h_sb[:, m, ts(blk, 128)],
                                        wdn_mh[:, mi, :],
                                        start=(m == 0), stop=False,
                                    )
                        for blk in range(CHS // 128):
                            nc.tensor.matmul(
                                psd[blk][:],
                                ones_sb[0:1, 0:128],
                                bdn_sb[0:1, ds(half * HW_, HW_)],
                                start=False, stop=True,
                            )
                            nc.vector.tensor_scalar(
                                y_parts[blk // 2][:, blk % 2, ds(half * HW_, HW_)],
                                psd[blk][:],
                                cw_sl[:, c * (CHS // 128) + blk:
                                      c * (CHS // 128) + blk + 1],
                                None,
                                op0=ALU.mult,
                            )
                    for yp in range(2):
                        nc.gpsimd.dma_scatter_add(
                            out_pad[:], y_parts[yp][:],
                            idx_rep[:, (c * 32 + yp * 16):(c * 32 + yp * 16 + 16)],
                            num_idxs=CHS // 2,
                            num_idxs_reg=CHS // 2, elem_size=D,
                        )

    nc.finalize()
    return nc


_NC_CACHE = None


def _get_program():
    global _NC_CACHE
    if _NC_CACHE is None:
        _NC_CACHE = build_program()
    return _NC_CACHE


def make_in_maps(hidden_states, gate_w, w_up, b_up, w_down, b_down):
    hidden_states = np.asarray(hidden_states, dtype=np.float32)
    gate_w = np.asarray(gate_w, dtype=np.float32)
    w_up = np.asarray(w_up, dtype=np.float32)
    b_up = np.asarray(b_up, dtype=np.float32)
    w_down = np.asarray(w_down, dtype=np.float32)
    b_down = np.asarray(b_down, dtype=np.float32)

    x = hidden_states.reshape(T, D)
    x_pad = np.zeros((TPAD, D), dtype=np.float32)
    x_pad[:T] = x
    xT_host = np.ascontiguousarray(x.T)
    ids = np.arange(T, dtype=np.float32).reshape(NT, 128).T.copy()  # [128, NT]
    ident = np.eye(128, dtype=np.float32)

    in_maps = []
    for c in range(E):
        gwc = np.concatenate([gate_w[:, c:], gate_w[:, :c]], axis=1).copy()
        wdn = w_down[c]  # [F, D]
        wdn_r = np.ascontiguousarray(
            wdn.reshape(FC, 128, 2, HW_).transpose(2, 0, 1, 3)
        ).reshape(2 * FC, 128, HW_)
        in_maps.append({
            "x_pad": x_pad,
            "xt": xT_host,
            "gwc": gwc,
            "wup": np.ascontiguousarray(w_up[c]),
            "bup": np.ascontiguousarray(b_up[c]),
            "wdn_r": wdn_r,
            "bdn": np.ascontiguousarray(b_down[c]),
            "ids": ids,
            "ident": ident,
            "ones": np.ones((1, 128), dtype=np.float32),
        })
    return in_maps


def combine_results(results):
    out = np.zeros((T, D), dtype=np.float32)
    for c in range(E):
        out += results[c]["out_pad"][:T]
    return out.reshape(B, S, D)


def kernel(hidden_states, gate_w, w_up, b_up, w_down, b_down):
    in_maps = make_in_maps(hidden_states, gate_w, w_up, b_up, w_down, b_down)
    nc = _get_program()
    res = run_bass_kernel_spmd(nc, in_maps, core_ids=list(range(E)))
    return combine_results(res.results)


if __name__ == "__main__":
    rng = np.random.default_rng(0)
    hs = rng.standard_normal((B, S, D)).astype(np.float32)
    gw = rng.standard_normal((D, E)).astype(np.float32) / np.sqrt(D)
    wu = (rng.standard_normal((E, D, F)) * 0.02).astype(np.float32)
    bu = np.zeros((E, F), dtype=np.float32)
    wd = (rng.standard_normal((E, F, D)) * 0.02).astype(np.float32)
    bd = np.zeros((E, D), dtype=np.float32)
    out = kernel(hs, gw, wu, bu, wd, bd)
    print("out", out.shape, out.dtype, np.abs(out).max())


# revision 17
# speedup vs baseline: 1.3988x; 1.0239x over previous
"""Trainium2 Bass kernel for nn_BertMoELayer (B=2,S=2048,D=768,F=3072,E=8,top-2).

Strategy: expert-parallel across 8 NeuronCores (1 expert per core).
Each core receives the full token set, computes the router (fp32), selects
the tokens routed to its expert (top-2 membership), compacts their indices
on-device (sparse_gather), gathers the token rows (dma_gather), runs the
expert FFN in fp32r (PE fast mode), scales by the combine weight, and
scatter-adds the rows into a zeroed output buffer (dma_scatter_add).
Host sums the 8 partial outputs.

Self-contained: hardcodes all shapes; only imports the installed concourse
stack from /opt/trn_rl_repo.
"""
import sys

sys.path.insert(0, "/opt/trn_rl_repo")

import numpy as np

import concourse.bass as bass
import concourse.tile as tile
from concourse import bacc, mybir
from concourse.bass import ds, ts
from concourse.bass_utils import run_bass_kernel_spmd

# Problem shapes
B, S, D, F, E = 2, 2048, 768, 3072, 8
T = B * S                 # 4096 tokens
CAP = 1536                # per-expert slot capacity (expected load 1024)
TPAD = T + CAP            # token rows incl. junk region for sentinel slots
DC = D // 128             # 6 contraction chunks for up-proj
FC = F // 128             # 24 contraction chunks for down-proj
NT = T // 128             # 32 token tiles
NCH = 3                   # FFN slot chunks
CHS = CAP // NCH          # 512 slots per chunk
NG = 8                    # gate groups of 512 tokens
CAND_F = (T + CAP) // 16  # 352 candidate free-dim
SENT_F = T // 16          # 256: sentinel region starts here
CAPF = CAP // 16          # 96
HW_ = D // 2              # 384: down-proj half width

F32 = mybir.dt.float32
F32R = mybir.dt.float32r
I16 = mybir.dt.int16
U32 = mybir.dt.uint32
ALU = mybir.AluOpType
AXX = mybir.AxisListType
ACT = mybir.ActivationFunctionType


def build_program():
    nc = bacc.Bacc("TRN2", target_bir_lowering=False, debug=False)

    x_pad = nc.dram_tensor("x_pad", (TPAD, D), F32, kind="ExternalInput")
    xt = nc.dram_tensor("xt", (D, T), F32, kind="ExternalInput")
    gwc = nc.dram_tensor("gwc", (D, E), F32, kind="ExternalInput")
    wup = nc.dram_tensor("wup", (D, F), F32R, kind="ExternalInput")
    bup = nc.dram_tensor("bup", (F,), F32, kind="ExternalInput")
    # wdn_r[half*FC + m] = w_down[m*128:(m+1)*128, half*384:(half+1)*384]
    wdn_r = nc.dram_tensor("wdn_r", (2 * FC, 128, HW_), F32R,
                           kind="ExternalInput")
    bdn = nc.dram_tensor("bdn", (D,), F32R, kind="ExternalInput")
    ids = nc.dram_tensor("ids", (128, NT), F32, kind="ExternalInput")
    ident = nc.dram_tensor("ident", (128, 128), F32, kind="ExternalInput")
    ones = nc.dram_tensor("ones", (1, 128), F32R, kind="ExternalInput")
    out_pad = nc.dram_tensor("out_pad", (TPAD, D), F32, kind="ExternalOutput")

    with tile.TileContext(nc) as tc:
        with (
            tc.tile_pool(name="const", bufs=1) as const_pool,
            tc.tile_pool(name="dram", bufs=1, space="DRAM") as dram_pool,
            tc.tile_pool(name="route", bufs=1) as route_pool,
        ):
            # ---- constants / small inputs ----
            ident_sb = const_pool.tile([128, 128], F32)
            nc.sync.dma_start(ident_sb[:], ident[:])
            gwc_sb = const_pool.tile([128, DC, E], F32)
            nc.sync.dma_start(gwc_sb[:], gwc.rearrange("(kc p) e -> p kc e", p=128))
            ids_sb = const_pool.tile([128, NT], F32)
            nc.sync.dma_start(ids_sb[:], ids[:])
            bup_sb = const_pool.tile([128, FC], F32)
            nc.scalar.dma_start(bup_sb[:], bup.rearrange("(m p) -> p m", p=128))
            bdn_sb = const_pool.tile([1, D], F32R)
            nc.scalar.dma_start(bdn_sb[:], bdn[None, :])
            ones_sb = const_pool.tile([1, 128], F32R)
            nc.scalar.dma_start(ones_sb[:], ones[:])

            # ---- resident up-proj weights (scalar rail; needed only at FFN) ----
            wup_sb = const_pool.tile([128, DC, F], F32R)
            nc.scalar.dma_start(wup_sb[:], wup.rearrange("(kc p) f -> p kc f", p=128))

            # ---- routing products (survive into the FFN phase) ----
            idx_rep = route_pool.tile([128, CAPF], I16)
            cw_sl = route_pool.tile([128, CAP // 128], F32)

            # =========== GATE PHASE ===========
            with (
                tc.tile_pool(name="gxt", bufs=3) as gxt_pool,
                tc.tile_pool(name="glt", bufs=2) as glt_pool,
                tc.tile_pool(name="gsoft", bufs=1) as gsoft_pool,
                tc.tile_pool(name="gps_tr", bufs=2, space="PSUM") as gps_tr,
                tc.tile_pool(name="gps_lt", bufs=2, space="PSUM") as gps_lt,
                tc.tile_pool(name="gps_ln", bufs=2, space="PSUM") as gps_ln,
            ):
                logits_sb = gsoft_pool.tile([128, NT, E], F32)
                for g in range(NG):
                    xT_g = gxt_pool.tile([128, DC, 512], F32, tag="xT")
                    nc.sync.dma_start(
                        xT_g[:],
                        xt[:, g * 512:(g + 1) * 512].rearrange(
                            "(kc p) t -> p kc t", p=128
                        ),
                    )
                    lps = gps_lt.tile([8, 512], F32, tag="lt")
                    for kc in range(DC):
                        nc.tensor.matmul(
                            lps[:], gwc_sb[:, kc, :], xT_g[:, kc, :],
                            start=(kc == 0), stop=(kc == DC - 1),
                        )
                    lT_sb = glt_pool.tile([8, 512], F32, tag="lT")
                    nc.any.tensor_copy(lT_sb[:], lps[:])
                    for j in range(4):
                        t = g * 4 + j
                        pn = gps_ln.tile([128, 8], F32, tag="ln")
                        nc.tensor.matmul(
                            pn[:], lT_sb[:, ts(j, 128)], ident_sb[0:8, 0:8]
                        )
                        nc.any.tensor_copy(logits_sb[:, t, :], pn[:])

                # ---- batched softmax + top-2 over all 32 tiles ----
                m1 = gsoft_pool.tile([128, NT], F32)
                nc.vector.tensor_reduce(m1[:], logits_sb[:], AXX.X, ALU.max)
                smx = gsoft_pool.tile([128, NT, E], F32)
                for e in range(E):
                    nc.vector.tensor_sub(
                        smx[:, :, e], logits_sb[:, :, e], m1[:]
                    )
                nc.scalar.activation(
                    smx[:].rearrange("p a b -> p (a b)"),
                    smx[:].rearrange("p a b -> p (a b)"), ACT.Exp,
                )
                zsum = gsoft_pool.tile([128, NT], F32)
                nc.vector.tensor_reduce(zsum[:], smx[:], AXX.X, ALU.add)
                rz = gsoft_pool.tile([128, NT], F32)
                nc.vector.reciprocal(rz[:], zsum[:])
                gt8 = gsoft_pool.tile([128, NT, E], F32)
                for e in range(E):
                    nc.vector.tensor_tensor(
                        gt8[:, :, e], logits_sb[:, :, e], logits_sb[:, :, 0],
                        op=ALU.is_gt,
                    )
                cnt = gsoft_pool.tile([128, NT], F32)
                nc.vector.tensor_reduce(cnt[:], gt8[:], AXX.X, ALU.add)
                mask = gsoft_pool.tile([128, NT], F32)
                nc.vector.tensor_scalar(mask[:], cnt[:], 1.5, None, op0=ALU.is_lt)
                mm1 = gsoft_pool.tile([128, NT], F32)
                nc.vector.tensor_scalar_add(mm1[:], mask[:], -1.0)
                cw0 = gsoft_pool.tile([128, NT], F32)
                nc.vector.tensor_tensor(cw0[:], smx[:, :, 0], rz[:], op=ALU.mult)
                cand_id = gsoft_pool.tile([128, NT], F32)
                cand_cw = gsoft_pool.tile([128, NT], F32)
                nc.vector.tensor_tensor(cand_cw[:], cw0[:], mask[:], op=ALU.mult)
                nc.vector.tensor_add(cand_cw[:], cand_cw[:], mm1[:])
                nc.vector.tensor_tensor(cand_id[:], ids_sb[:], mask[:], op=ALU.mult)
                nc.vector.tensor_add(cand_id[:], cand_id[:], mm1[:])

                # ---- compaction ----
                # regroup [128,32] -> [16,256] via PE transpose (any candidate
                # order works; only "sentinels last" matters)
                cand16_id = gsoft_pool.tile([16, CAND_F], F32)
                cand16_cw = gsoft_pool.tile([16, CAND_F], F32)
                for cbuf, c16 in ((cand_id, cand16_id), (cand_cw, cand16_cw)):
                    pct = gps_tr.tile([32, 128], F32, tag="tr")
                    nc.tensor.matmul(pct[:], cbuf[:], ident_sb[:])
                    ctT = gsoft_pool.tile([32, 128], F32, tag="ctT")
                    nc.any.tensor_copy(ctT[:], pct[:])
                    nc.vector.tensor_copy(c16[:, 0:128], ctT[0:16, :])
                    nc.gpsimd.dma_start(c16[:, 128:256], ctT[16:32, :])
                # sentinel candidates: token T (junk row), weight 0
                nc.any.memset(cand16_id[:, SENT_F:CAND_F], float(T))
                nc.any.memset(cand16_cw[:, SENT_F:CAND_F], 0.0)

                # output sized = input so compaction can never overflow; only
                # the first CAPF free-columns (1536 slots) are used downstream.
                sg_id = gsoft_pool.tile([16, CAND_F], F32)
                sg_cw = route_pool.tile([16, CAND_F], F32)
                nf1 = gsoft_pool.tile([1, 1], U32)
                nf2 = route_pool.tile([1, 1], U32)
                nc.gpsimd.sparse_gather(sg_id[:], cand16_id[:], num_found=nf1[:])
                nc.gpsimd.sparse_gather(sg_cw[:], cand16_cw[:], num_found=nf2[:])

                # int16 + replicate to all 8 16-partition groups
                nc.vector.tensor_copy(idx_rep[0:16, :], sg_id[:, 0:CAPF])
                nc.gpsimd.dma_start(idx_rep[16:32, :], idx_rep[0:16, :])
                nc.gpsimd.dma_start(idx_rep[32:64, :], idx_rep[0:32, :])
                nc.gpsimd.dma_start(idx_rep[64:128, :], idx_rep[0:64, :])


            # =========== FFN PHASE ===========
            with (
                tc.tile_pool(name="fxg", bufs=2) as fxg_pool,
                tc.tile_pool(name="fmisc", bufs=1) as fmisc_pool,
                tc.tile_pool(name="fxt", bufs=2) as fxt_pool,
                tc.tile_pool(name="fh", bufs=1) as fh_pool,
                tc.tile_pool(name="fwd", bufs=6) as fwd_pool,
                tc.tile_pool(name="fy", bufs=1) as fy_pool,
                tc.tile_pool(name="fps_tr", bufs=2, space="PSUM") as fps_tr,
                tc.tile_pool(name="fps_up", bufs=2, space="PSUM") as fps_up,
                tc.tile_pool(name="fps_dn", bufs=4, space="PSUM") as fps_dn,
            ):
                for c in range(NCH):
                    idx_c = idx_rep[:, c * (CHS // 16):(c + 1) * (CHS // 16)]
                    xg = fxg_pool.tile([128, CHS // 128, D], F32, tag="xg")
                    nc.gpsimd.dma_gather(
                        xg[:], x_pad[:], idx_c, num_idxs=CHS,
                        num_idxs_reg=CHS, elem_size=D,
                    )
                    xcT = fxt_pool.tile([128, DC, CHS], F32R, tag="xcT")
                    for j in range(CHS // 128):
                        for kc in range(DC):
                            pt = fps_tr.tile([128, 128], F32, tag="tr")
                            nc.tensor.matmul(
                                pt[:], xg[:, j, ts(kc, 128)], ident_sb[:]
                            )
                            nc.any.tensor_copy(
                                xcT[:, kc, ds(j * 128, 128)], pt[:]
                            )
                    # up-projection + gelu -> h^T [128, FC, CHS]
                    h_sb = fh_pool.tile([128, FC, CHS], F32R, tag="h")
                    for m in range(FC):
                        psu = fps_up.tile([128, CHS], F32, tag="up")
                        for kc in range(DC):
                            nc.tensor.matmul(
                                psu[:],
                                wup_sb[:, kc, ts(m, 128)],
                                xcT[:, kc, :],
                                start=(kc == 0), stop=(kc == DC - 1),
                            )
                        nc.scalar.activation(
                            h_sb[:, m, :], psu[:], ACT.Gelu,
                            bias=bup_sb[:, m:m + 1],
                        )
                    if c == 0:
                        # combine weights (s%16, s//16) -> slot-major [128,12]:
                        # [16,96] -T-> [96,16] -> DRAM -> [12,128] -T-> [128,12]
                        pcw = fps_tr.tile([96, 16], F32, tag="tr")
                        nc.tensor.matmul(pcw[:], sg_cw[:, 0:CAPF],
                                         ident_sb[0:16, 0:16])
                        cwT = fmisc_pool.tile([96, 16], F32)
                        nc.any.tensor_copy(cwT[:], pcw[:])
                        scr_cw2 = dram_pool.tile([CAP], F32, tag="scr_cw2")
                        nc.gpsimd.dma_start(
                            scr_cw2[:].rearrange("(f b) -> f b", b=16), cwT[:]
                        )
                        cw12 = fmisc_pool.tile([12, 128], F32)
                        nc.gpsimd.dma_start(
                            cw12[:], scr_cw2[:].rearrange("(j p) -> j p", p=128)
                        )
                        pcw2 = fps_tr.tile([128, 12], F32, tag="tr")
                        nc.tensor.matmul(pcw2[:], cw12[:], ident_sb[0:12, 0:12])
                        nc.any.tensor_copy(cw_sl[:], pcw2[:])
                    # down-projection, natural orientation, + bias + scale
                    y_lo = fy_pool.tile([128, 2, D], F32, tag="ylo")
                    y_hi = fy_pool.tile([128, 2, D], F32, tag="yhi")
                    y_parts = [y_lo, y_hi]
                    for half in range(2):
                        psd = []
                        for _pi in range(CHS // 128):
                            psd_t = fps_dn.tile([128, HW_], F32, tag="dn")
                            psd.append(psd_t)
                        for m2 in range(FC // 2):
                            wdn_mh = fwd_pool.tile([128, 2, HW_], F32R, tag="wdn")
                            nc.scalar.dma_start(
                                wdn_mh[:],
                                wdn_r[half * FC + 2 * m2:half * FC + 2 * m2 + 2,
                                      :, :].rearrange("two p h -> p two h"),
                            )
                            for mi in range(2):
                                m = 2 * m2 + mi
                                for blk in range(CHS // 128):
                                    nc.tensor.matmul(
                                        psd[blk][:],
                                        h_sb[:, m, ts(blk, 128)],
                                        wdn_mh[:, mi, :],
                                        start=(m == 0), stop=False,
                                    )
                        for blk in range(CHS // 128):
                            nc.tensor.matmul(
                                psd[blk][:],
                                ones_sb[0:1, 0:128],
                                bdn_sb[0:1, ds(half * HW_, HW_)],
                                start=False, stop=True,
                            )
                            nc.vector.tensor_scalar(
                                y_parts[blk // 2][:, blk % 2, ds(half * HW_, HW_)],
                                psd[blk][:],
                                cw_sl[:, c * (CHS // 128) + blk:
                                      c * (CHS // 128) + blk + 1],
                                None,
                                op0=ALU.mult,
                            )
                    for yp in range(2):
                        nc.gpsimd.dma_scatter_add(
                            out_pad[:], y_parts[yp][:],
                            idx_rep[:, (c * 32 + yp * 16):(c * 32 + yp * 16 + 16)],
                            num_idxs=CHS // 2,
                            num_idxs_reg=CHS // 2, elem_size=D,
                        )

    nc.finalize()
    return nc


_NC_CACHE = None


def _get_program():
    global _NC_CACHE
    if _NC_CACHE is None:
        _NC_CACHE = build_program()
    return _NC_CACHE


def make_in_maps(hidden_states, gate_w, w_up, b_up, w_down, b_down):
    hidden_states = np.asarray(hidden_states, dtype=np.float32)
    gate_w = np.asarray(gate_w, dtype=np.float32)
    w_up = np.asarray(w_up, dtype=np.float32)
    b_up = np.asarray(b_up, dtype=np.float32)
    w_down = np.asarray(w_down, dtype=np.float32)
    b_down = np.asarray(b_down, dtype=np.float32)

    x = hidden_states.reshape(T, D)
    x_pad = np.zeros((TPAD, D), dtype=np.float32)
    x_pad[:T] = x
    xT_host = np.ascontiguousarray(x.T)
    ids = np.arange(T, dtype=np.float32).reshape(NT, 128).T.copy()  # [128, NT]
    ident = np.eye(128, dtype=np.float32)

    in_maps = []
    for c in range(E):
        gwc = np.concatenate([gate_w[:, c:], gate_w[:, :c]], axis=1).copy()
        wdn = w_down[c]  # [F, D]
        wdn_r = np.ascontiguousarray(
            wdn.reshape(FC, 128, 2, HW_).transpose(2, 0, 1, 3)
        ).reshape(2 * FC, 128, HW_)
        in_maps.append({
            "x_pad": x_pad,
            "xt": xT_host,
            "gwc": gwc,
            "wup": np.ascontiguousarray(w_up[c]),
            "bup": np.ascontiguousarray(b_up[c]),
            "wdn_r": wdn_r,
            "bdn": np.ascontiguousarray(b_down[c]),
            "ids": ids,
            "ident": ident,
            "ones": np.ones((1, 128), dtype=np.float32),
        })
    return in_maps


def combine_results(results):
    out = np.zeros((T, D), dtype=np.float32)
    for c in range(E):
        out += results[c]["out_pad"][:T]
    return out.reshape(B, S, D)


def kernel(hidden_states, gate_w, w_up, b_up, w_down, b_down):
    in_maps = make_in_maps(hidden_states, gate_w, w_up, b_up, w_down, b_down)
    nc = _get_program()
    res = run_bass_kernel_spmd(nc, in_maps, core_ids=list(range(E)))
    return combine_results(res.results)


if __name__ == "__main__":
    rng = np.random.default_rng(0)
    hs = rng.standard_normal((B, S, D)).astype(np.float32)
    gw = rng.standard_normal((D, E)).astype(np.float32) / np.sqrt(D)
    wu = (rng.standard_normal((E, D, F)) * 0.02).astype(np.float32)
    bu = np.zeros((E, F), dtype=np.float32)
    wd = (rng.standard_normal((E, F, D)) * 0.02).astype(np.float32)
    bd = np.zeros((E, D), dtype=np.float32)
    out = kernel(hs, gw, wu, bu, wd, bd)
    print("out", out.shape, out.dtype, np.abs(out).max())


# revision 18
# speedup vs baseline: 1.4631x; 1.0460x over previous
"""Trainium2 Bass kernel for nn_BertMoELayer (B=2,S=2048,D=768,F=3072,E=8,top-2).

Strategy: expert-parallel across 8 NeuronCores (1 expert per core).
Each core receives the full token set, computes the router (fp32), selects
the tokens routed to its expert (top-2 membership), compacts their indices
on-device (sparse_gather), gathers the token rows (dma_gather), runs the
expert FFN in fp32r (PE fast mode), scales by the combine weight, and
scatter-adds the rows into a zeroed output buffer (dma_scatter_add).
Host sums the 8 partial outputs.

Self-contained: hardcodes all shapes; only imports the installed concourse
stack from /opt/trn_rl_repo.
"""
import sys

sys.path.insert(0, "/opt/trn_rl_repo")

import numpy as np

import concourse.bass as bass
import concourse.tile as tile
from concourse import bacc, mybir
from concourse.bass import ds, ts
from concourse.bass_utils import run_bass_kernel_spmd

# Problem shapes
B, S, D, F, E = 2, 2048, 768, 3072, 8
T = B * S                 # 4096 tokens
CAP = 1536                # per-expert slot capacity (expected load 1024)
TPAD = T + CAP            # token rows incl. junk region for sentinel slots
DC = D // 128             # 6 contraction chunks for up-proj
FC = F // 128             # 24 contraction chunks for down-proj
NT = T // 128             # 32 token tiles
NCH = 3                   # FFN slot chunks
CHS = CAP // NCH          # 512 slots per chunk
NG = 8                    # gate groups of 512 tokens
CAND_F = (T + CAP) // 16  # 352 candidate free-dim
SENT_F = T // 16          # 256: sentinel region starts here
CAPF = CAP // 16          # 96
HW_ = D // 2              # 384: down-proj half width

F32 = mybir.dt.float32
F32R = mybir.dt.float32r
I16 = mybir.dt.int16
U32 = mybir.dt.uint32
ALU = mybir.AluOpType
AXX = mybir.AxisListType
ACT = mybir.ActivationFunctionType


def build_program():
    nc = bacc.Bacc("TRN2", target_bir_lowering=False, debug=False)

    x_pad = nc.dram_tensor("x_pad", (TPAD, D), F32, kind="ExternalInput")
    xt = nc.dram_tensor("xt", (D, T), F32, kind="ExternalInput")
    gwc = nc.dram_tensor("gwc", (D, E), F32, kind="ExternalInput")
    wup = nc.dram_tensor("wup", (D, F), F32R, kind="ExternalInput")
    bup = nc.dram_tensor("bup", (F,), F32, kind="ExternalInput")
    # wdn_r[half*FC + m] = w_down[m*128:(m+1)*128, half*384:(half+1)*384]
    wdn_r = nc.dram_tensor("wdn_r", (2 * FC, 128, HW_), F32R,
                           kind="ExternalInput")
    bdn = nc.dram_tensor("bdn", (D,), F32R, kind="ExternalInput")
    ids = nc.dram_tensor("ids", (128, NT), F32, kind="ExternalInput")
    ident = nc.dram_tensor("ident", (128, 128), F32, kind="ExternalInput")
    ones = nc.dram_tensor("ones", (1, 128), F32R, kind="ExternalInput")
    out_pad = nc.dram_tensor("out_pad", (TPAD, D), F32, kind="ExternalOutput")

    with tile.TileContext(nc) as tc:
        with (
            tc.tile_pool(name="const", bufs=1) as const_pool,
            tc.tile_pool(name="dram", bufs=1, space="DRAM") as dram_pool,
            tc.tile_pool(name="route", bufs=1) as route_pool,
        ):
            # ---- constants / small inputs ----
            ident_sb = const_pool.tile([128, 128], F32)
            nc.sync.dma_start(ident_sb[:], ident[:])
            gwc_sb = const_pool.tile([128, DC, E], F32)
            nc.sync.dma_start(gwc_sb[:], gwc.rearrange("(kc p) e -> p kc e", p=128))
            ids_sb = const_pool.tile([128, NT], F32)
            nc.sync.dma_start(ids_sb[:], ids[:])
            bup_sb = const_pool.tile([128, FC], F32)
            nc.scalar.dma_start(bup_sb[:], bup.rearrange("(m p) -> p m", p=128))
            bdn_sb = const_pool.tile([1, D], F32R)
            nc.scalar.dma_start(bdn_sb[:], bdn[None, :])
            ones_sb = const_pool.tile([1, 128], F32R)
            nc.scalar.dma_start(ones_sb[:], ones[:])

            # ---- resident up-proj weights (loaded after the gate xT loads) ----
            wup_sb = const_pool.tile([128, DC, F], F32R)

            # ---- routing products (survive into the FFN phase) ----
            idx_rep = route_pool.tile([128, CAPF], I16)
            cw_sl = route_pool.tile([128, CAP // 128], F32)

            # =========== GATE PHASE ===========
            with (
                tc.tile_pool(name="gxt", bufs=3) as gxt_pool,
                tc.tile_pool(name="glt", bufs=2) as glt_pool,
                tc.tile_pool(name="gsoft", bufs=1) as gsoft_pool,
                tc.tile_pool(name="gps_tr", bufs=2, space="PSUM") as gps_tr,
                tc.tile_pool(name="gps_lt", bufs=2, space="PSUM") as gps_lt,
                tc.tile_pool(name="gps_ln", bufs=2, space="PSUM") as gps_ln,
            ):
                logits_sb = gsoft_pool.tile([128, NT, E], F32)
                for g in range(NG):
                    xT_g = gxt_pool.tile([128, DC, 512], F32, tag="xT")
                    nc.sync.dma_start(
                        xT_g[:],
                        xt[:, g * 512:(g + 1) * 512].rearrange(
                            "(kc p) t -> p kc t", p=128
                        ),
                    )
                    lps = gps_lt.tile([8, 512], F32, tag="lt")
                    for kc in range(DC):
                        nc.tensor.matmul(
                            lps[:], gwc_sb[:, kc, :], xT_g[:, kc, :],
                            start=(kc == 0), stop=(kc == DC - 1),
                        )
                    lT_sb = glt_pool.tile([8, 512], F32, tag="lT")
                    nc.any.tensor_copy(lT_sb[:], lps[:])
                    for j in range(4):
                        t = g * 4 + j
                        pn = gps_ln.tile([128, 8], F32, tag="ln")
                        nc.tensor.matmul(
                            pn[:], lT_sb[:, ts(j, 128)], ident_sb[0:8, 0:8]
                        )
                        nc.any.tensor_copy(logits_sb[:, t, :], pn[:])

                nc.sync.dma_start(
                    wup_sb[:], wup.rearrange("(kc p) f -> p kc f", p=128)
                )

                # ---- batched softmax + top-2 over all 32 tiles ----
                m1 = gsoft_pool.tile([128, NT], F32)
                nc.vector.tensor_reduce(m1[:], logits_sb[:], AXX.X, ALU.max)
                smx = gsoft_pool.tile([128, NT, E], F32)
                for e in range(E):
                    nc.vector.tensor_sub(
                        smx[:, :, e], logits_sb[:, :, e], m1[:]
                    )
                nc.scalar.activation(
                    smx[:].rearrange("p a b -> p (a b)"),
                    smx[:].rearrange("p a b -> p (a b)"), ACT.Exp,
                )
                zsum = gsoft_pool.tile([128, NT], F32)
                nc.vector.tensor_reduce(zsum[:], smx[:], AXX.X, ALU.add)
                rz = gsoft_pool.tile([128, NT], F32)
                nc.vector.reciprocal(rz[:], zsum[:])
                gt8 = gsoft_pool.tile([128, NT, E], F32)
                for e in range(E):
                    nc.vector.tensor_tensor(
                        gt8[:, :, e], logits_sb[:, :, e], logits_sb[:, :, 0],
                        op=ALU.is_gt,
                    )
                cnt = gsoft_pool.tile([128, NT], F32)
                nc.vector.tensor_reduce(cnt[:], gt8[:], AXX.X, ALU.add)
                mask = gsoft_pool.tile([128, NT], F32)
                nc.vector.tensor_scalar(mask[:], cnt[:], 1.5, None, op0=ALU.is_lt)
                mm1 = gsoft_pool.tile([128, NT], F32)
                nc.vector.tensor_scalar_add(mm1[:], mask[:], -1.0)
                cw0 = gsoft_pool.tile([128, NT], F32)
                nc.vector.tensor_tensor(cw0[:], smx[:, :, 0], rz[:], op=ALU.mult)
                cand_id = gsoft_pool.tile([128, NT], F32)
                cand_cw = gsoft_pool.tile([128, NT], F32)
                nc.vector.tensor_tensor(cand_cw[:], cw0[:], mask[:], op=ALU.mult)
                nc.vector.tensor_add(cand_cw[:], cand_cw[:], mm1[:])
                nc.vector.tensor_tensor(cand_id[:], ids_sb[:], mask[:], op=ALU.mult)
                nc.vector.tensor_add(cand_id[:], cand_id[:], mm1[:])

                # ---- compaction ----
                # regroup [128,32] -> [16,256] via PE transpose (any candidate
                # order works; only "sentinels last" matters)
                cand16_id = gsoft_pool.tile([16, CAND_F], F32)
                cand16_cw = gsoft_pool.tile([16, CAND_F], F32)
                for cbuf, c16 in ((cand_id, cand16_id), (cand_cw, cand16_cw)):
                    pct = gps_tr.tile([32, 128], F32, tag="tr")
                    nc.tensor.matmul(pct[:], cbuf[:], ident_sb[:])
                    ctT = gsoft_pool.tile([32, 128], F32, tag="ctT")
                    nc.any.tensor_copy(ctT[:], pct[:])
                    nc.vector.tensor_copy(c16[:, 0:128], ctT[0:16, :])
                    nc.gpsimd.dma_start(c16[:, 128:256], ctT[16:32, :])
                # sentinel candidates: token T (junk row), weight 0
                nc.any.memset(cand16_id[:, SENT_F:CAND_F], float(T))
                nc.any.memset(cand16_cw[:, SENT_F:CAND_F], 0.0)

                # output sized = input so compaction can never overflow; only
                # the first CAPF free-columns (1536 slots) are used downstream.
                sg_id = gsoft_pool.tile([16, CAND_F], F32)
                sg_cw = route_pool.tile([16, CAND_F], F32)
                nf1 = gsoft_pool.tile([1, 1], U32)
                nf2 = route_pool.tile([1, 1], U32)
                nc.gpsimd.sparse_gather(sg_id[:], cand16_id[:], num_found=nf1[:])
                sg_cw_inst = nc.gpsimd.sparse_gather(
                    sg_cw[:], cand16_cw[:], num_found=nf2[:]
                )

                # int16 + replicate to all 8 16-partition groups
                nc.vector.tensor_copy(idx_rep[0:16, :], sg_id[:, 0:CAPF])
                nc.gpsimd.dma_start(idx_rep[16:32, :], idx_rep[0:16, :])
                nc.gpsimd.dma_start(idx_rep[32:64, :], idx_rep[0:32, :])
                nc.gpsimd.dma_start(idx_rep[64:128, :], idx_rep[0:64, :])


            # =========== FFN PHASE ===========
            with (
                tc.tile_pool(name="fxg", bufs=2) as fxg_pool,
                tc.tile_pool(name="fmisc", bufs=1) as fmisc_pool,
                tc.tile_pool(name="fxt", bufs=2) as fxt_pool,
                tc.tile_pool(name="fh", bufs=1) as fh_pool,
                tc.tile_pool(name="fwd", bufs=6) as fwd_pool,
                tc.tile_pool(name="fy", bufs=1) as fy_pool,
                tc.tile_pool(name="fps_tr", bufs=2, space="PSUM") as fps_tr,
                tc.tile_pool(name="fps_up", bufs=2, space="PSUM") as fps_up,
                tc.tile_pool(name="fps_dn", bufs=4, space="PSUM") as fps_dn,
            ):
                for c in range(NCH):
                    idx_c = idx_rep[:, c * (CHS // 16):(c + 1) * (CHS // 16)]
                    xg = fxg_pool.tile([128, CHS // 128, D], F32, tag="xg")
                    g_inst = nc.gpsimd.dma_gather(
                        xg[:], x_pad[:], idx_c, num_idxs=CHS,
                        num_idxs_reg=CHS, elem_size=D,
                    )
                    if c == 0:
                        tile.add_dep_helper(
                            g_inst.ins, sg_cw_inst.ins, sync=False,
                            reason="group library-8 gpsimd ops before library-3",
                        )
                    xcT = fxt_pool.tile([128, DC, CHS], F32R, tag="xcT")
                    for j in range(CHS // 128):
                        for kc in range(DC):
                            pt = fps_tr.tile([128, 128], F32, tag="tr")
                            nc.tensor.matmul(
                                pt[:], xg[:, j, ts(kc, 128)], ident_sb[:]
                            )
                            nc.any.tensor_copy(
                                xcT[:, kc, ds(j * 128, 128)], pt[:]
                            )
                    # up-projection + gelu -> h^T [128, FC, CHS]
                    h_sb = fh_pool.tile([128, FC, CHS], F32R, tag="h")
                    for m in range(FC):
                        psu = fps_up.tile([128, CHS], F32, tag="up")
                        for kc in range(DC):
                            nc.tensor.matmul(
                                psu[:],
                                wup_sb[:, kc, ts(m, 128)],
                                xcT[:, kc, :],
                                start=(kc == 0), stop=(kc == DC - 1),
                            )
                        nc.scalar.activation(
                            h_sb[:, m, :], psu[:], ACT.Gelu,
                            bias=bup_sb[:, m:m + 1],
                        )
                    if c == 0:
                        # combine weights (s%16, s//16) -> slot-major [128,12]:
                        # [16,96] -T-> [96,16] -> DRAM -> [12,128] -T-> [128,12]
                        pcw = fps_tr.tile([96, 16], F32, tag="tr")
                        nc.tensor.matmul(pcw[:], sg_cw[:, 0:CAPF],
                                         ident_sb[0:16, 0:16])
                        cwT = fmisc_pool.tile([96, 16], F32)
                        nc.any.tensor_copy(cwT[:], pcw[:])
                        scr_cw2 = dram_pool.tile([CAP], F32, tag="scr_cw2")
                        nc.gpsimd.dma_start(
                            scr_cw2[:].rearrange("(f b) -> f b", b=16), cwT[:]
                        )
                        cw12 = fmisc_pool.tile([12, 128], F32)
                        nc.gpsimd.dma_start(
                            cw12[:], scr_cw2[:].rearrange("(j p) -> j p", p=128)
                        )
                        pcw2 = fps_tr.tile([128, 12], F32, tag="tr")
                        nc.tensor.matmul(pcw2[:], cw12[:], ident_sb[0:12, 0:12])
                        nc.any.tensor_copy(cw_sl[:], pcw2[:])
                    # down-projection, natural orientation, + bias + scale
                    y_lo = fy_pool.tile([128, 2, D], F32, tag="ylo")
                    y_hi = fy_pool.tile([128, 2, D], F32, tag="yhi")
                    y_parts = [y_lo, y_hi]
                    for half in range(2):
                        psd = []
                        for _pi in range(CHS // 128):
                            psd_t = fps_dn.tile([128, HW_], F32, tag="dn")
                            psd.append(psd_t)
                        for m2 in range(FC // 2):
                            wdn_mh = fwd_pool.tile([128, 2, HW_], F32R, tag="wdn")
                            nc.scalar.dma_start(
                                wdn_mh[:],
                                wdn_r[half * FC + 2 * m2:half * FC + 2 * m2 + 2,
                                      :, :].rearrange("two p h -> p two h"),
                            )
                            for mi in range(2):
                                m = 2 * m2 + mi
                                for blk in range(CHS // 128):
                                    nc.tensor.matmul(
                                        psd[blk][:],
                                        h_sb[:, m, ts(blk, 128)],
                                        wdn_mh[:, mi, :],
                                        start=(m == 0), stop=False,
                                    )
                        for blk in range(CHS // 128):
                            nc.tensor.matmul(
                                psd[blk][:],
                                ones_sb[0:1, 0:128],
                                bdn_sb[0:1, ds(half * HW_, HW_)],
                                start=False, stop=True,
                            )
                            nc.vector.tensor_scalar(
                                y_parts[blk // 2][:, blk % 2, ds(half * HW_, HW_)],
                                psd[blk][:],
                                cw_sl[:, c * (CHS // 128) + blk:
                                      c * (CHS // 128) + blk + 1],
                                None,
                                op0=ALU.mult,
                            )
                    for yp in range(2):
                        nc.gpsimd.dma_scatter_add(
                            out_pad[:], y_parts[yp][:],
                            idx_rep[:, (c * 32 + yp * 16):(c * 32 + yp * 16 + 16)],
                            num_idxs=CHS // 2,
                            num_idxs_reg=CHS // 2, elem_size=D,
                        )

    nc.finalize()
    return nc


_NC_CACHE = None


def _get_program():
    global _NC_CACHE
    if _NC_CACHE is None:
        _NC_CACHE = build_program()
    return _NC_CACHE


def make_in_maps(hidden_states, gate_w, w_up, b_up, w_down, b_down):
    hidden_states = np.asarray(hidden_states, dtype=np.float32)
    gate_w = np.asarray(gate_w, dtype=np.float32)
    w_up = np.asarray(w_up, dtype=np.float32)
    b_up = np.asarray(b_up, dtype=np.float32)
    w_down = np.asarray(w_down, dtype=np.float32)
    b_down = np.asarray(b_down, dtype=np.float32)

    x = hidden_states.reshape(T, D)
    x_pad = np.zeros((TPAD, D), dtype=np.float32)
    x_pad[:T] = x
    xT_host = np.ascontiguousarray(x.T)
    ids = np.arange(T, dtype=np.float32).reshape(NT, 128).T.copy()  # [128, NT]
    ident = np.eye(128, dtype=np.float32)

    in_maps = []
    for c in range(E):
        gwc = np.concatenate([gate_w[:, c:], gate_w[:, :c]], axis=1).copy()
        wdn = w_down[c]  # [F, D]
        wdn_r = np.ascontiguousarray(
            wdn.reshape(FC, 128, 2, HW_).transpose(2, 0, 1, 3)
        ).reshape(2 * FC, 128, HW_)
        in_maps.append({
            "x_pad": x_pad,
            "xt": xT_host,
            "gwc": gwc,
            "wup": np.ascontiguousarray(w_up[c]),
            "bup": np.ascontiguousarray(b_up[c]),
            "wdn_r": wdn_r,
            "bdn": np.ascontiguousarray(b_down[c]),
            "ids": ids,
            "ident": ident,
            "ones": np.ones((1, 128), dtype=np.float32),
        })
    return in_maps


def combine_results(results):
    out = np.zeros((T, D), dtype=np.float32)
    for c in range(E):
        out += results[c]["out_pad"][:T]
    return out.reshape(B, S, D)


def kernel(hidden_states, gate_w, w_up, b_up, w_down, b_down):
    in_maps = make_in_maps(hidden_states, gate_w, w_up, b_up, w_down, b_down)
    nc = _get_program()
    res = run_bass_kernel_spmd(nc, in_maps, core_ids=list(range(E)))
    return combine_results(res.results)


if __name__ == "__main__":
    rng = np.random.default_rng(0)
    hs = rng.standard_normal((B, S, D)).astype(np.float32)
    gw = rng.standard_normal((D, E)).astype(np.float32) / np.sqrt(D)
    wu = (rng.standard_normal((E, D, F)) * 0.02).astype(np.float32)
    bu = np.zeros((E, F), dtype=np.float32)
    wd = (rng.standard_normal((E, F, D)) * 0.02).astype(np.float32)
    bd = np.zeros((E, D), dtype=np.float32)
    out = kernel(hs, gw, wu, bu, wd, bd)
    print("out", out.shape, out.dtype, np.abs(out).max())
